# revision 3
# baseline (speedup 1.0000x reference)
"""Trainium2 Bass kernel for nn_BaseAttention (B=4, H=16, S=2048, D=64, key-mask).

Strategy (8 NeuronCores, batch*head sharded, 8 heads per core):
  For each head (Q,K,V: [S,D] f32, mask: [S] int 0/1):
    - Load Q,K,V with fp32->bf16 cast during DMA (SWDGE).
    - PE-transpose Q,K tiles -> Q^T, K^T [D=64, S] bf16 in SBUF.
    - Scores transposed: S^T[k, q] = K @ Q^T via matmul(lhsT=K^T chunk, rhs=Q^T),
      fp32 PSUM.  Softmax needs exp over k (partition axis in this layout), so
      mask+scale+exp fuse into ONE ScalarE pass:
          P^T = Exp(S^T * (1/sqrt(D)) + bias[k]),  bias[k] = -1e4 * mask[k]
      (per-partition bias).  No max-subtraction: scores are ~N(0,1), exp is safe
      in fp32, masked entries underflow to 0 exactly like the reference.
    - Denominator for free: V' = [V | ones] (M=64->65 doesn't change stream
      length), out'^T[0:64, q] = unnormalized out^T, out'^T[64, q] = sum(exp).
    - Reciprocal of the sums row, PE-transpose [65, q] -> [q, 65], multiply by
      per-partition recip, store.

Self-contained: hardcodes shapes; imports concourse from /opt/trn_rl_repo.
"""

import sys

if "/opt/trn_rl_repo" not in sys.path:
    sys.path.insert(0, "/opt/trn_rl_repo")

import numpy as np

import concourse.bass as bass
import concourse.mybir as mybir
import concourse.tile as tile
from concourse import bacc
from concourse.masks import make_identity

F32 = mybir.dt.float32
BF16 = mybir.dt.bfloat16
I32 = mybir.dt.int32

N_CORES = 8
B, NH, S, D = 4, 16, 2048, 64
H = (B * NH) // N_CORES  # heads per core = 8
P = 128                  # partitions / k-tile size
T = S // P               # 16 k-tiles per head
W = 1024                 # q-window width
NW = S // W              # 2 q-windows per head
QC = 512                 # matmul moving chunk (fp32 PSUM out limit)
SCALE = 1.0 / 8.0        # 1/sqrt(D)
NEG = -10000.0


def emit_core_program(ctx, nc, tc, q_h, k_h, v_h, mask_h, out_h):
    """Emit the per-core Tile program. q/k/v/out: DRAM APs [H, S, D]; mask: [S] i32."""
    pool = lambda *a, **kw: ctx.enter_context(tc.tile_pool(*a, **kw))
    singles = pool(name="singles", bufs=1)
    ld = pool(name="ld", bufs=2)            # SBUF head staging (bf16)
    qkT = pool(name="qkT", bufs=2)          # SBUF Q^T/K^T
    ppool = pool(name="p", bufs=3)          # SBUF P^T tiles
    accs_pool = pool(name="accs", bufs=2)   # SBUF drained accumulators
    outs_pool = pool(name="outs", bufs=2)   # SBUF output staging
    st_pool = pool(name="st", bufs=2, space="PSUM")    # S^T tiles (2 banks ea)
    acc_pool = pool(name="acc", bufs=1, space="PSUM")  # out'^T accum (2 banks)
    tp_pool = pool(name="tp", bufs=2, space="PSUM")    # transposes (1 bank ea)

    ident_bf = singles.tile([P, P], BF16)
    make_identity(nc, ident_bf)
    ident_f32 = singles.tile([P, P], F32)
    make_identity(nc, ident_f32)

    # mask [S] i32 -> bias [128, T] f32 = -1e4 * mask, bias[p, t] = key t*128+p
    mask_i = singles.tile([P, T], I32)
    nc.sync.dma_start(out=mask_i, in_=mask_h.rearrange("(t p) -> p t", p=P))
    bias = singles.tile([P, T], F32)
    nc.vector.tensor_scalar_mul(bias, mask_i, NEG)

    for h in range(H):
        # ---- load + cast (SWDGE casts fp32 -> bf16 in flight) ----
        q_sb = ld.tile([P, T, D], BF16, tag="q_sb")
        nc.gpsimd.dma_start(out=q_sb, in_=q_h[h].rearrange("(t p) d -> p t d", p=P))
        k_sb = ld.tile([P, T, D], BF16, tag="k_sb")
        nc.gpsimd.dma_start(out=k_sb, in_=k_h[h].rearrange("(t p) d -> p t d", p=P))
        v_sb = ld.tile([P, T, D + 1], BF16, tag="v_sb")
        nc.gpsimd.dma_start(
            out=v_sb[:, :, 0:D], in_=v_h[h].rearrange("(t p) d -> p t d", p=P)
        )
        nc.vector.memset(v_sb[:, :, D : D + 1], 1.0)

        # ---- transpose Q, K -> [64, S] bf16 ----
        qT = qkT.tile([D, S], BF16, tag="qT")
        kT = qkT.tile([D, S], BF16, tag="kT")
        for src, dst in ((q_sb, qT), (k_sb, kT)):
            for g in range(T // 4):
                tp = tp_pool.tile([D, 4 * P], BF16, tag="tp")
                for j in range(4):
                    nc.tensor.transpose(
                        tp[:, j * P : (j + 1) * P], src[:, 4 * g + j, :], ident_bf
                    )
                nc.vector.tensor_copy(dst[:, 4 * g * P : 4 * (g + 1) * P], tp)

        # ---- attention over q-windows ----
        for w in range(NW):
            q0 = w * W
            acc = acc_pool.tile([D + 1, W], F32, tag="acc")
            for t in range(T):
                st = st_pool.tile([P, W], F32, tag="st")
                for c in range(W // QC):
                    nc.tensor.matmul(
                        st[:, c * QC : (c + 1) * QC],
                        lhsT=kT[:, t * P : (t + 1) * P],
                        rhs=qT[:, q0 + c * QC : q0 + (c + 1) * QC],
                        start=True,
                        stop=True,
                    )
                pT = ppool.tile([P, W], BF16, tag="pT")
                nc.scalar.activation(
                    out=pT,
                    in_=st,
                    func=mybir.ActivationFunctionType.Exp,
                    bias=bias[:, t : t + 1],
                    scale=SCALE,
                )
                for c in range(W // QC):
                    nc.tensor.matmul(
                        acc[:, c * QC : (c + 1) * QC],
                        lhsT=v_sb[:, t, :],
                        rhs=pT[:, c * QC : (c + 1) * QC],
                        start=(t == 0),
                        stop=(t == T - 1),
                    )

            # ---- drain accum, reciprocal of sums row, transpose, scale, store ----
            accs = accs_pool.tile([D + 1, W], F32, tag="accs")
            nc.vector.tensor_copy(accs, acc)
            nc.vector.reciprocal(accs[D : D + 1, :], accs[D : D + 1, :])
            ost = outs_pool.tile([P, W // P, D], F32, tag="ost")
            for j in range(W // P):
                ot = tp_pool.tile([P, D + 1], F32, tag="tp")
                nc.tensor.transpose(
                    ot, accs[:, j * P : (j + 1) * P], ident_f32[: D + 1, : D + 1]
                )
                nc.vector.tensor_scalar_mul(ost[:, j, :], ot[:, 0:D], ot[:, D : D + 1])
            nc.sync.dma_start(
                out=out_h[h, q0 : q0 + W, :].rearrange("(j p) d -> p j d", p=P),
                in_=ost,
            )


def build_nc():
    nc = bacc.Bacc("TRN2", target_bir_lowering=False, debug=False, num_devices=N_CORES)
    q = nc.declare_dram_parameter("q", [H, S, D], F32, isOutput=False)
    k = nc.declare_dram_parameter("k", [H, S, D], F32, isOutput=False)
    v = nc.declare_dram_parameter("v", [H, S, D], F32, isOutput=False)
    mask = nc.declare_dram_parameter("mask", [S], I32, isOutput=False)
    out = nc.declare_dram_parameter("out", [H, S, D], F32, isOutput=True)
    from contextlib import ExitStack

    with tile.TileContext(nc) as tc, ExitStack() as ctx:
        emit_core_program(ctx, nc, tc, q.ap(), k.ap(), v.ap(), mask.ap(), out.ap())
    nc.compile()
    return nc


_NC_CACHE = []


def get_nc():
    if not _NC_CACHE:
        _NC_CACHE.append(build_nc())
    return _NC_CACHE[0]


def make_in_maps(q, k, v, mask):
    """Shard full [B,NH,S,D] inputs into per-core input maps (8 heads/core)."""
    qf = np.asarray(q, dtype=np.float32).reshape(B * NH, S, D)
    kf = np.asarray(k, dtype=np.float32).reshape(B * NH, S, D)
    vf = np.asarray(v, dtype=np.float32).reshape(B * NH, S, D)
    mf = np.asarray(mask, dtype=np.int32).reshape(B, S)
    in_maps = []
    for c in range(N_CORES):
        lo = c * H
        in_maps.append(
            {
                "q": np.ascontiguousarray(qf[lo : lo + H]),
                "k": np.ascontiguousarray(kf[lo : lo + H]),
                "v": np.ascontiguousarray(vf[lo : lo + H]),
                # heads lo..lo+H-1 all belong to batch lo // NH
                "mask": np.ascontiguousarray(mf[lo // NH]),
            }
        )
    return in_maps


def kernel(q, k, v, mask):
    from concourse.bass_utils import run_bass_kernel_spmd

    nc = get_nc()
    in_maps = make_in_maps(q, k, v, mask)
    res = run_bass_kernel_spmd(nc, in_maps, list(range(N_CORES))).results
    out = np.concatenate([res[c]["out"] for c in range(N_CORES)], axis=0)
    return out.reshape(B, NH, S, D)


if __name__ == "__main__":
    nc = build_nc()
    print("built ok")


# revision 8
# speedup vs baseline: 1.7743x; 1.7743x over previous
"""Trainium2 Bass kernel for nn_BaseAttention (B=4, H=16, S=2048, D=64, key-mask).

Strategy (8 NeuronCores, batch*head sharded, 8 heads per core):
  For each head (Q,K,V: [S,D] f32, mask: [S] int 0/1):
    - Load Q,K,V with fp32->bf16 cast during DMA (SWDGE).
    - PE-transpose Q,K tiles -> Q^T, K^T [D=64, S] bf16 in SBUF.
    - Scores transposed: S^T[k, q] = K @ Q^T via matmul(lhsT=K^T chunk, rhs=Q^T),
      fp32 PSUM.  Softmax needs exp over k (partition axis in this layout), so
      mask+scale+exp fuse into ONE ScalarE pass:
          P^T = Exp(S^T * (1/sqrt(D)) + bias[k]),  bias[k] = -1e4 * mask[k]
      (per-partition bias).  No max-subtraction: scores are ~N(0,1), exp is safe
      in fp32, masked entries underflow to 0 exactly like the reference.
    - Denominator for free: V' = [V | ones] (M=64->65 doesn't change stream
      length), out'^T[0:64, q] = unnormalized out^T, out'^T[64, q] = sum(exp).
    - Reciprocal of the sums row, PE-transpose [65, q] -> [q, 65], multiply by
      per-partition recip, store.

Self-contained: hardcodes shapes; imports concourse from /opt/trn_rl_repo.
"""

import sys

if "/opt/trn_rl_repo" not in sys.path:
    sys.path.insert(0, "/opt/trn_rl_repo")

import numpy as np

import concourse.bass as bass
import concourse.mybir as mybir
import concourse.tile as tile
from concourse import bacc
from concourse.masks import make_identity

F32 = mybir.dt.float32
BF16 = mybir.dt.bfloat16
I32 = mybir.dt.int32

N_CORES = 8
B, NH, S, D = 4, 16, 2048, 64
H = (B * NH) // N_CORES  # heads per core = 8
P = 128                  # partitions / k-tile size
T = S // P               # 16 k-tiles per head
W = 1024                 # q-window width
NW = S // W              # 2 q-windows per head
QC = 512                 # matmul moving chunk (fp32 PSUM out limit)
SCALE = 1.0 / 8.0        # 1/sqrt(D)
NEG = -10000.0


def emit_core_program(ctx, nc, tc, q_h, k_h, v_h, mask_h, out_h):
    """Emit the per-core Tile program. q/k/v/out: DRAM APs [H, S, D]; mask: [S] i32."""
    pool = lambda *a, **kw: ctx.enter_context(tc.tile_pool(*a, **kw))
    singles = pool(name="singles", bufs=1)
    ld = pool(name="ld", bufs=2)            # SBUF head staging (bf16)
    qkT = pool(name="qkT", bufs=2)          # SBUF Q^T/K^T
    ppool = pool(name="p", bufs=3)          # SBUF P^T tiles
    accs_pool = pool(name="accs", bufs=2)   # SBUF drained accumulators
    outs_pool = pool(name="outs", bufs=2)   # SBUF output staging
    st_pool = pool(name="st", bufs=2, space="PSUM")    # S^T tiles (2 banks ea)
    acc_pool = pool(name="acc", bufs=1, space="PSUM")  # out'^T accum (2 banks)
    tp_pool = pool(name="tp", bufs=2, space="PSUM")    # transposes (1 bank ea)

    ident_bf = singles.tile([P, P], BF16)
    make_identity(nc, ident_bf)
    ident_f32 = singles.tile([P, P], F32)
    make_identity(nc, ident_f32)

    # mask [S] i32 -> bias [128, T] f32 = -1e4 * mask, bias[p, t] = key t*128+p
    mask_i = singles.tile([P, T], I32)
    nc.sync.dma_start(out=mask_i, in_=mask_h.rearrange("(t p) -> p t", p=P))
    bias = singles.tile([P, T], F32)
    nc.vector.tensor_scalar_mul(bias, mask_i, NEG)

    # -------- software-pipelined emission --------
    # Per-engine streams are in-order, so emission order decides overlap:
    #  * mm2 lags mm1/exp by one k-tile (PE never waits on the exp it just fed)
    #  * window epilogue (transpose/recip/scale/store) is deferred into the
    #    next window's k-loop; the accumulator drain happens immediately so
    #    the single PSUM acc slot frees fast
    #  * next head's loads + Q/K transposes are emitted mid-window

    def emit_head_load(h):
        q_sb = ld.tile([P, T, D], BF16, tag="q_sb")
        nc.gpsimd.dma_start(out=q_sb, in_=q_h[h].rearrange("(t p) d -> p t d", p=P))
        k_sb = ld.tile([P, T, D], BF16, tag="k_sb")
        nc.gpsimd.dma_start(out=k_sb, in_=k_h[h].rearrange("(t p) d -> p t d", p=P))
        v_sb = ld.tile([P, T, D + 1], BF16, tag="v_sb")
        nc.gpsimd.dma_start(
            out=v_sb[:, :, 0:D], in_=v_h[h].rearrange("(t p) d -> p t d", p=P)
        )
        nc.vector.memset(v_sb[:, :, D : D + 1], 1.0)
        return q_sb, k_sb, v_sb

    def emit_head_transpose(q_sb, k_sb):
        qT = qkT.tile([D, S], BF16, tag="qT")
        kT = qkT.tile([D, S], BF16, tag="kT")
        for src, dst in ((q_sb, qT), (k_sb, kT)):
            for g in range(T // 4):
                tp = tp_pool.tile([D, 4 * P], BF16, tag="tp")
                for j in range(4):
                    nc.tensor.transpose(
                        tp[:, j * P : (j + 1) * P], src[:, 4 * g + j, :], ident_bf
                    )
                nc.vector.tensor_copy(dst[:, 4 * g * P : 4 * (g + 1) * P], tp)
        return qT, kT

    def emit_epilogue_rest(ep):
        # transpose [65, W] -> W/P tiles of [q=128, 65], normalize, store
        h, q0, accs = ep
        ost = outs_pool.tile([P, W // P, D], F32, tag="ost")
        for j in range(W // P):
            ot = tp_pool.tile([P, D + 1], F32, tag="tp")
            nc.tensor.transpose(
                ot, accs[:, j * P : (j + 1) * P], ident_f32[: D + 1, : D + 1]
            )
            nc.vector.reciprocal(ot[:, D : D + 1], ot[:, D : D + 1])
            nc.vector.tensor_scalar_mul(ost[:, j, :], ot[:, 0:D], ot[:, D : D + 1])
        nc.sync.dma_start(
            out=out_h[h, q0 : q0 + W, :].rearrange("(j p) d -> p j d", p=P),
            in_=ost,
        )

    heads = {0: emit_head_load(0)}
    headsT = {0: emit_head_transpose(*heads[0][:2])}
    pending_epilogue = None
    passes = [(h, w) for h in range(H) for w in range(NW)]
    for h, w in passes:
        if w == 0 and h > 0:
            del heads[h - 1], headsT[h - 1]
        qT, kT = headsT[h]
        v_sb = heads[h][2]
        q0 = w * W
        acc = acc_pool.tile([D + 1, W], F32, tag="acc")
        pTs = {}
        for t in range(T):
            st = st_pool.tile([P, W], F32, tag="st")
            for c in range(W // QC):
                nc.tensor.matmul(
                    st[:, c * QC : (c + 1) * QC],
                    lhsT=kT[:, t * P : (t + 1) * P],
                    rhs=qT[:, q0 + c * QC : q0 + (c + 1) * QC],
                    start=True,
                    stop=True,
                )
            pT = ppool.tile([P, W], BF16, tag="pT")
            nc.scalar.activation(
                out=pT,
                in_=st,
                func=mybir.ActivationFunctionType.Exp,
                bias=bias[:, t : t + 1],
                scale=SCALE,
            )
            pTs[t] = pT
            if t >= 1:  # mm2 for previous k-tile
                pT_prev = pTs.pop(t - 1)
                for c in range(W // QC):
                    nc.tensor.matmul(
                        acc[:, c * QC : (c + 1) * QC],
                        lhsT=v_sb[:, t - 1, :],
                        rhs=pT_prev[:, c * QC : (c + 1) * QC],
                        start=(t - 1 == 0),
                        stop=False,
                    )
            if t == 1 and pending_epilogue is not None:
                emit_epilogue_rest(pending_epilogue)
                pending_epilogue = None
            if t == 4 and w == 0 and h + 1 < H:
                heads[h + 1] = emit_head_load(h + 1)
            if t == 4 and w == NW - 1 and h + 1 < H:
                headsT[h + 1] = emit_head_transpose(*heads[h + 1][:2])
        pT_last = pTs.pop(T - 1)
        for c in range(W // QC):
            nc.tensor.matmul(
                acc[:, c * QC : (c + 1) * QC],
                lhsT=v_sb[:, T - 1, :],
                rhs=pT_last[:, c * QC : (c + 1) * QC],
                start=False,
                stop=True,
            )
        # drain accumulator now (frees the single PSUM acc slot); rest deferred
        accs = accs_pool.tile([D + 1, W], F32, tag="accs")
        nc.vector.tensor_copy(accs, acc)
        pending_epilogue = (h, q0, accs)
    emit_epilogue_rest(pending_epilogue)


def build_nc():
    nc = bacc.Bacc("TRN2", target_bir_lowering=False, debug=False, num_devices=N_CORES)
    q = nc.declare_dram_parameter("q", [H, S, D], F32, isOutput=False)
    k = nc.declare_dram_parameter("k", [H, S, D], F32, isOutput=False)
    v = nc.declare_dram_parameter("v", [H, S, D], F32, isOutput=False)
    mask = nc.declare_dram_parameter("mask", [S], I32, isOutput=False)
    out = nc.declare_dram_parameter("out", [H, S, D], F32, isOutput=True)
    from contextlib import ExitStack

    with tile.TileContext(nc) as tc, ExitStack() as ctx:
        emit_core_program(ctx, nc, tc, q.ap(), k.ap(), v.ap(), mask.ap(), out.ap())
    nc.compile()
    return nc


_NC_CACHE = []


def get_nc():
    if not _NC_CACHE:
        _NC_CACHE.append(build_nc())
    return _NC_CACHE[0]


def make_in_maps(q, k, v, mask):
    """Shard full [B,NH,S,D] inputs into per-core input maps (8 heads/core)."""
    qf = np.asarray(q, dtype=np.float32).reshape(B * NH, S, D)
    kf = np.asarray(k, dtype=np.float32).reshape(B * NH, S, D)
    vf = np.asarray(v, dtype=np.float32).reshape(B * NH, S, D)
    mf = np.asarray(mask, dtype=np.int32).reshape(B, S)
    in_maps = []
    for c in range(N_CORES):
        lo = c * H
        in_maps.append(
            {
                "q": np.ascontiguousarray(qf[lo : lo + H]),
                "k": np.ascontiguousarray(kf[lo : lo + H]),
                "v": np.ascontiguousarray(vf[lo : lo + H]),
                # heads lo..lo+H-1 all belong to batch lo // NH
                "mask": np.ascontiguousarray(mf[lo // NH]),
            }
        )
    return in_maps


def kernel(q, k, v, mask):
    from concourse.bass_utils import run_bass_kernel_spmd

    nc = get_nc()
    in_maps = make_in_maps(q, k, v, mask)
    res = run_bass_kernel_spmd(nc, in_maps, list(range(N_CORES))).results
    out = np.concatenate([res[c]["out"] for c in range(N_CORES)], axis=0)
    return out.reshape(B, NH, S, D)


if __name__ == "__main__":
    nc = build_nc()
    print("built ok")


# revision 12
# speedup vs baseline: 1.8510x; 1.0432x over previous
"""Trainium2 Bass kernel for nn_BaseAttention (B=4, H=16, S=2048, D=64, key-mask).

Strategy (8 NeuronCores, batch*head sharded, 8 heads per core):
  For each head (Q,K,V: [S,D] f32, mask: [S] int 0/1):
    - Load Q,K,V with fp32->bf16 cast during DMA (SWDGE).
    - PE-transpose Q,K tiles -> Q^T, K^T [D=64, S] bf16 in SBUF.
    - Scores transposed: S^T[k, q] = K @ Q^T via matmul(lhsT=K^T chunk, rhs=Q^T),
      fp32 PSUM.  Softmax needs exp over k (partition axis in this layout), so
      mask+scale+exp fuse into ONE ScalarE pass:
          P^T = Exp(S^T * (1/sqrt(D)) + bias[k]),  bias[k] = -1e4 * mask[k]
      (per-partition bias).  No max-subtraction: scores are ~N(0,1), exp is safe
      in fp32, masked entries underflow to 0 exactly like the reference.
    - Denominator for free: V' = [V | ones] (M=64->65 doesn't change stream
      length), out'^T[0:64, q] = unnormalized out^T, out'^T[64, q] = sum(exp).
    - Reciprocal of the sums row, PE-transpose [65, q] -> [q, 65], multiply by
      per-partition recip, store.

Self-contained: hardcodes shapes; imports concourse from /opt/trn_rl_repo.
"""

import sys

if "/opt/trn_rl_repo" not in sys.path:
    sys.path.insert(0, "/opt/trn_rl_repo")

import numpy as np

import concourse.bass as bass
import concourse.mybir as mybir
import concourse.tile as tile
from concourse import bacc
from concourse.masks import make_identity

F32 = mybir.dt.float32
BF16 = mybir.dt.bfloat16
I32 = mybir.dt.int32

N_CORES = 8
B, NH, S, D = 4, 16, 2048, 64
H = (B * NH) // N_CORES  # heads per core = 8
P = 128                  # partitions / k-tile size
T = S // P               # 16 k-tiles per head
W = 1024                 # q-window width
NW = S // W              # 2 q-windows per head
QC = 512                 # matmul moving chunk (fp32 PSUM out limit)
SCALE = 1.0 / 8.0        # 1/sqrt(D)
NEG = -10000.0


def emit_core_program(ctx, nc, tc, q_h, k_h, v_h, mask_h, out_h):
    """Emit the per-core Tile program. q/k/v/out: DRAM APs [H, S, D]; mask: [S] i32."""
    pool = lambda *a, **kw: ctx.enter_context(tc.tile_pool(*a, **kw))
    singles = pool(name="singles", bufs=1)
    ld = pool(name="ld", bufs=2)            # SBUF head staging (bf16)
    qkT = pool(name="qkT", bufs=2)          # SBUF Q^T/K^T
    ppool = pool(name="p", bufs=5)          # SBUF P^T tiles (pair-lagged mm2)
    accs_pool = pool(name="accs", bufs=2)   # SBUF drained accumulators
    outs_pool = pool(name="outs", bufs=2)   # SBUF output staging
    st_pool = pool(name="st", bufs=2, space="PSUM")    # S^T tiles (2 banks ea)
    acc_pool = pool(name="acc", bufs=1, space="PSUM")  # out'^T accum (2 banks)
    tp_pool = pool(name="tp", bufs=2, space="PSUM")    # transposes (1 bank ea)

    ident_bf = singles.tile([P, P], BF16)
    make_identity(nc, ident_bf)
    ident_f32 = singles.tile([P, P], F32)
    make_identity(nc, ident_f32)

    # mask [S] i32 -> bias [128, T] f32 = -1e4 * mask, bias[p, t] = key t*128+p
    mask_i = singles.tile([P, T], I32)
    nc.sync.dma_start(out=mask_i, in_=mask_h.rearrange("(t p) -> p t", p=P))
    bias = singles.tile([P, T], F32)
    nc.vector.tensor_scalar_mul(bias, mask_i, NEG)

    # -------- software-pipelined emission --------
    # Per-engine streams are in-order, so emission order decides overlap:
    #  * mm2 lags mm1/exp by one k-tile (PE never waits on the exp it just fed)
    #  * window epilogue (transpose/recip/scale/store) is deferred into the
    #    next window's k-loop; the accumulator drain happens immediately so
    #    the single PSUM acc slot frees fast
    #  * next head's loads + Q/K transposes are emitted mid-window

    def emit_head_load(h):
        q_sb = ld.tile([P, T, D], BF16, tag="q_sb")
        nc.gpsimd.dma_start(out=q_sb, in_=q_h[h].rearrange("(t p) d -> p t d", p=P))
        k_sb = ld.tile([P, T, D], BF16, tag="k_sb")
        nc.gpsimd.dma_start(out=k_sb, in_=k_h[h].rearrange("(t p) d -> p t d", p=P))
        v_sb = ld.tile([P, T, D + 1], BF16, tag="v_sb")
        nc.gpsimd.dma_start(
            out=v_sb[:, :, 0:D], in_=v_h[h].rearrange("(t p) d -> p t d", p=P)
        )
        nc.vector.memset(v_sb[:, :, D : D + 1], 1.0)
        return q_sb, k_sb, v_sb

    def emit_head_transpose(q_sb, k_sb):
        # Q^T / K^T [64, S], then duplicate onto partitions 64-127 (SBUF->SBUF
        # DMA) so mm1 can run two k-tiles concurrently in the two PE row halves.
        qT = qkT.tile([2 * D, S], BF16, tag="qT")
        kT = qkT.tile([2 * D, S], BF16, tag="kT")
        for src, dst in ((q_sb, qT), (k_sb, kT)):
            for g in range(T // 4):
                tp = tp_pool.tile([D, 4 * P], BF16, tag="tp")
                for j in range(4):
                    nc.tensor.transpose(
                        tp[:, j * P : (j + 1) * P], src[:, 4 * g + j, :], ident_bf
                    )
                nc.vector.tensor_copy(dst[:D, 4 * g * P : 4 * (g + 1) * P], tp)
            nc.sync.dma_start(out=dst[D : 2 * D, :], in_=dst[:D, :])
        return qT, kT

    def emit_epilogue_rest(ep):
        # transpose [65, W] -> W/P tiles of [q=128, 65], normalize, store
        h, q0, accs = ep
        ost = outs_pool.tile([P, W // P, D], F32, tag="ost")
        for j in range(W // P):
            ot = tp_pool.tile([P, D + 1], F32, tag="tp")
            nc.tensor.transpose(
                ot, accs[:, j * P : (j + 1) * P], ident_f32[: D + 1, : D + 1]
            )
            nc.vector.reciprocal(ot[:, D : D + 1], ot[:, D : D + 1])
            nc.vector.tensor_scalar_mul(ost[:, j, :], ot[:, 0:D], ot[:, D : D + 1])
        nc.sync.dma_start(
            out=out_h[h, q0 : q0 + W, :].rearrange("(j p) d -> p j d", p=P),
            in_=ost,
        )

    heads = {0: emit_head_load(0)}
    headsT = {0: emit_head_transpose(*heads[0][:2])}
    pending_epilogue = None
    passes = [(h, w) for h in range(H) for w in range(NW)]
    for h, w in passes:
        if w == 0 and h > 0:
            del heads[h - 1], headsT[h - 1]
        qT, kT = headsT[h]
        v_sb = heads[h][2]
        q0 = w * W
        acc = acc_pool.tile([D + 1, W], F32, tag="acc")
        pTs = {}

        def emit_mm2(t, last):
            pT_prev = pTs.pop(t)
            for c in range(W // QC):
                nc.tensor.matmul(
                    acc[:, c * QC : (c + 1) * QC],
                    lhsT=v_sb[:, t, :],
                    rhs=pT_prev[:, c * QC : (c + 1) * QC],
                    start=(t == 0),
                    stop=last,
                )

        for j in range(T // 2):  # k-tile pairs: (2j) on PE rows 0-63, (2j+1) on 64-127
            t0, t1 = 2 * j, 2 * j + 1
            st0 = st_pool.tile([P, W], F32, tag="st")
            st1 = st_pool.tile([P, W], F32, tag="st")
            for c in range(W // QC):
                for t, st, lo in ((t0, st0, 0), (t1, st1, D)):
                    nc.tensor.matmul(
                        st[:, c * QC : (c + 1) * QC],
                        lhsT=kT[lo : lo + D, t * P : (t + 1) * P],
                        rhs=qT[lo : lo + D, q0 + c * QC : q0 + (c + 1) * QC],
                        start=True,
                        stop=True,
                    )
            for t, st in ((t0, st0), (t1, st1)):
                pT = ppool.tile([P, W], BF16, tag="pT")
                nc.scalar.activation(
                    out=pT,
                    in_=st,
                    func=mybir.ActivationFunctionType.Exp,
                    bias=bias[:, t : t + 1],
                    scale=SCALE,
                )
                pTs[t] = pT
            if j >= 1:  # mm2 for previous k-tile pair
                emit_mm2(t0 - 2, False)
                emit_mm2(t1 - 2, False)
            if j == 1 and pending_epilogue is not None:
                emit_epilogue_rest(pending_epilogue)
                pending_epilogue = None
            if j == 2 and w == 0 and h + 1 < H:
                heads[h + 1] = emit_head_load(h + 1)
            if j == 2 and w == NW - 1 and h + 1 < H:
                headsT[h + 1] = emit_head_transpose(*heads[h + 1][:2])
        emit_mm2(T - 2, False)
        emit_mm2(T - 1, True)
        # drain accumulator now (frees the single PSUM acc slot); rest deferred
        accs = accs_pool.tile([D + 1, W], F32, tag="accs")
        nc.vector.tensor_copy(accs, acc)
        pending_epilogue = (h, q0, accs)
    emit_epilogue_rest(pending_epilogue)


def build_nc():
    nc = bacc.Bacc("TRN2", target_bir_lowering=False, debug=False, num_devices=N_CORES)
    q = nc.declare_dram_parameter("q", [H, S, D], F32, isOutput=False)
    k = nc.declare_dram_parameter("k", [H, S, D], F32, isOutput=False)
    v = nc.declare_dram_parameter("v", [H, S, D], F32, isOutput=False)
    mask = nc.declare_dram_parameter("mask", [S], I32, isOutput=False)
    out = nc.declare_dram_parameter("out", [H, S, D], F32, isOutput=True)
    from contextlib import ExitStack

    with tile.TileContext(nc) as tc, ExitStack() as ctx:
        emit_core_program(ctx, nc, tc, q.ap(), k.ap(), v.ap(), mask.ap(), out.ap())
    nc.compile()
    return nc


_NC_CACHE = []


def get_nc():
    if not _NC_CACHE:
        _NC_CACHE.append(build_nc())
    return _NC_CACHE[0]


def make_in_maps(q, k, v, mask):
    """Shard full [B,NH,S,D] inputs into per-core input maps (8 heads/core)."""
    qf = np.asarray(q, dtype=np.float32).reshape(B * NH, S, D)
    kf = np.asarray(k, dtype=np.float32).reshape(B * NH, S, D)
    vf = np.asarray(v, dtype=np.float32).reshape(B * NH, S, D)
    mf = np.asarray(mask, dtype=np.int32).reshape(B, S)
    in_maps = []
    for c in range(N_CORES):
        lo = c * H
        in_maps.append(
            {
                "q": np.ascontiguousarray(qf[lo : lo + H]),
                "k": np.ascontiguousarray(kf[lo : lo + H]),
                "v": np.ascontiguousarray(vf[lo : lo + H]),
                # heads lo..lo+H-1 all belong to batch lo // NH
                "mask": np.ascontiguousarray(mf[lo // NH]),
            }
        )
    return in_maps


def kernel(q, k, v, mask):
    from concourse.bass_utils import run_bass_kernel_spmd

    nc = get_nc()
    in_maps = make_in_maps(q, k, v, mask)
    res = run_bass_kernel_spmd(nc, in_maps, list(range(N_CORES))).results
    out = np.concatenate([res[c]["out"] for c in range(N_CORES)], axis=0)
    return out.reshape(B, NH, S, D)


if __name__ == "__main__":
    nc = build_nc()
    print("built ok")


# revision 17
# speedup vs baseline: 1.8704x; 1.0104x over previous
"""Trainium2 Bass kernel for nn_BaseAttention (B=4, H=16, S=2048, D=64, key-mask).

Strategy (8 NeuronCores, batch*head sharded, 8 heads per core):
  For each head (Q,K,V: [S,D] f32, mask: [S] int 0/1):
    - Load Q,K,V with fp32->bf16 cast during DMA (SWDGE).
    - PE-transpose Q,K tiles -> Q^T, K^T [D=64, S] bf16 in SBUF.
    - Scores transposed: S^T[k, q] = K @ Q^T via matmul(lhsT=K^T chunk, rhs=Q^T),
      fp32 PSUM.  Softmax needs exp over k (partition axis in this layout), so
      mask+scale+exp fuse into ONE ScalarE pass:
          P^T = Exp(S^T * (1/sqrt(D)) + bias[k]),  bias[k] = -1e4 * mask[k]
      (per-partition bias).  No max-subtraction: scores are ~N(0,1), exp is safe
      in fp32, masked entries underflow to 0 exactly like the reference.
    - Denominator for free: V' = [V | ones] (M=64->65 doesn't change stream
      length), out'^T[0:64, q] = unnormalized out^T, out'^T[64, q] = sum(exp).
    - Reciprocal of the sums row, PE-transpose [65, q] -> [q, 65], multiply by
      per-partition recip, store.

Self-contained: hardcodes shapes; imports concourse from /opt/trn_rl_repo.
"""

import sys

if "/opt/trn_rl_repo" not in sys.path:
    sys.path.insert(0, "/opt/trn_rl_repo")

import numpy as np

import concourse.bass as bass
import concourse.mybir as mybir
import concourse.tile as tile
from concourse import bacc
from concourse.masks import make_identity

F32 = mybir.dt.float32
BF16 = mybir.dt.bfloat16
I32 = mybir.dt.int32

N_CORES = 8
B, NH, S, D = 4, 16, 2048, 64
H = (B * NH) // N_CORES  # heads per core = 8
P = 128                  # partitions / k-tile size
T = S // P               # 16 k-tiles per head
W = 512                  # q-window width (= matmul moving chunk, fp32 PSUM out limit)
NW = S // W              # 4 q-windows per head
SCALE = 1.0 / 8.0        # 1/sqrt(D)
NEG = -10000.0


def emit_core_program(ctx, nc, tc, q_h, k_h, v_h, mask_h, out_h):
    """Emit the per-core Tile program. q/k/v/out: DRAM APs [H, S, D]; mask: [S] i32."""
    pool = lambda *a, **kw: ctx.enter_context(tc.tile_pool(*a, **kw))
    singles = pool(name="singles", bufs=1)
    ld = pool(name="ld", bufs=2)            # SBUF head staging (bf16)
    qkT = pool(name="qkT", bufs=2)          # SBUF Q^T/K^T
    ppool = pool(name="p", bufs=5)          # SBUF P^T tiles (pair-lagged mm2)
    accs_pool = pool(name="accs", bufs=2)   # SBUF drained accumulators
    outs_pool = pool(name="outs", bufs=2)   # SBUF output staging
    st_pool = pool(name="st", bufs=2, space="PSUM")    # S^T pair tiles (2 banks ea)
    acc_pool = pool(name="acc", bufs=2, space="PSUM")  # out'^T accum (1 bank ea)
    tp_pool = pool(name="tp", bufs=2, space="PSUM")    # transposes (1 bank ea)

    ident_bf = singles.tile([P, P], BF16)
    make_identity(nc, ident_bf)
    ident_f32 = singles.tile([P, P], F32)
    make_identity(nc, ident_f32)

    # mask [S] i32 -> om [128, T] f32 = 1 - mask (om[p, t] = key t*128+p kept?)
    # The mask is applied by zeroing masked rows of V' (incl. the ones column):
    # out = sum_k exp(s_k) V'[k] makes that exactly equivalent to score masking.
    mask_i = singles.tile([P, T], I32)
    nc.sync.dma_start(out=mask_i, in_=mask_h.rearrange("(t p) -> p t", p=P))
    om = singles.tile([P, T], F32)
    nc.vector.tensor_scalar(om, mask_i, -1.0, 1.0, mybir.AluOpType.mult, mybir.AluOpType.add)

    # -------- software-pipelined emission --------
    # Per-engine streams are in-order, so emission order decides overlap:
    #  * mm2 lags mm1/exp by one k-tile (PE never waits on the exp it just fed)
    #  * window epilogue (transpose/recip/scale/store) is deferred into the
    #    next window's k-loop; the accumulator drain happens immediately so
    #    the single PSUM acc slot frees fast
    #  * next head's loads + Q/K transposes are emitted mid-window

    def emit_head_load(h):
        q_sb = ld.tile([P, T, D], BF16, tag="q_sb")
        nc.gpsimd.dma_start(out=q_sb, in_=q_h[h].rearrange("(t p) d -> p t d", p=P))
        k_sb = ld.tile([P, T, D], BF16, tag="k_sb")
        nc.gpsimd.dma_start(out=k_sb, in_=k_h[h].rearrange("(t p) d -> p t d", p=P))
        v_sb = ld.tile([P, T, D + 1], BF16, tag="v_sb")
        nc.gpsimd.dma_start(
            out=v_sb[:, :, 0:D], in_=v_h[h].rearrange("(t p) d -> p t d", p=P)
        )
        nc.vector.memset(v_sb[:, :, D : D + 1], 1.0)
        for t in range(T):  # zero masked key rows of V' (applies the mask)
            nc.vector.tensor_scalar_mul(v_sb[:, t, :], v_sb[:, t, :], om[:, t : t + 1])
        return q_sb, k_sb, v_sb

    def emit_head_transpose(q_sb, k_sb):
        # Q^T / K^T [64, S], then duplicate onto partitions 64-127 (SBUF->SBUF
        # DMA) so mm1 can run two k-tiles concurrently in the two PE row halves.
        qT = qkT.tile([2 * D, S], BF16, tag="qT")
        kT = qkT.tile([2 * D, S], BF16, tag="kT")
        for src, dst in ((q_sb, qT), (k_sb, kT)):
            for g in range(T // 4):
                tp = tp_pool.tile([D, 4 * P], BF16, tag="tp")
                for j in range(4):
                    nc.tensor.transpose(
                        tp[:, j * P : (j + 1) * P], src[:, 4 * g + j, :], ident_bf
                    )
                nc.vector.tensor_copy(dst[:D, 4 * g * P : 4 * (g + 1) * P], tp)
            nc.sync.dma_start(out=dst[D : 2 * D, :], in_=dst[:D, :])
        return qT, kT

    def emit_epilogue_rest(ep):
        # transpose [65, W] -> W/P tiles of [q=128, 65], normalize, store
        h, q0, accs = ep
        ost = outs_pool.tile([P, W // P, D], F32, tag="ost")
        for j in range(W // P):
            ot = tp_pool.tile([P, D + 1], F32, tag="tp")
            nc.tensor.transpose(
                ot, accs[:, j * P : (j + 1) * P], ident_f32[: D + 1, : D + 1]
            )
            nc.vector.reciprocal(ot[:, D : D + 1], ot[:, D : D + 1])
            nc.vector.tensor_scalar_mul(ost[:, j, :], ot[:, 0:D], ot[:, D : D + 1])
        nc.sync.dma_start(
            out=out_h[h, q0 : q0 + W, :].rearrange("(j p) d -> p j d", p=P),
            in_=ost,
        )

    heads = {0: emit_head_load(0)}
    headsT = {0: emit_head_transpose(*heads[0][:2])}
    pending_epilogue = None
    passes = [(h, w) for h in range(H) for w in range(NW)]
    for h, w in passes:
        if w == 0 and h > 0:
            del heads[h - 1], headsT[h - 1]
        qT, kT = headsT[h]
        v_sb = heads[h][2]
        q0 = w * W
        acc = acc_pool.tile([D + 1, W], F32, tag="acc")
        pTs = {}

        def emit_mm2(j, last):
            pT_prev = pTs.pop(j)
            for c, t in ((0, 2 * j), (1, 2 * j + 1)):
                nc.tensor.matmul(
                    acc,
                    lhsT=v_sb[:, t, :],
                    rhs=pT_prev[:, c * W : (c + 1) * W],
                    start=(j == 0 and c == 0),
                    stop=last and (c == 1),
                )

        for j in range(T // 2):  # k-tile pair: (2j) on PE rows 0-63, (2j+1) on 64-127
            # one PSUM tile holds S^T for both k-tiles of the pair side by side,
            # written by two concurrently-executing row-tiled matmuls
            st = st_pool.tile([P, 2 * W], F32, tag="st")
            for c, (t, lo) in enumerate(((2 * j, 0), (2 * j + 1, D))):
                nc.tensor.matmul(
                    st[:, c * W : (c + 1) * W],
                    lhsT=kT[lo : lo + D, t * P : (t + 1) * P],
                    rhs=qT[lo : lo + D, q0 : q0 + W],
                    start=True,
                    stop=True,
                )
            pT = ppool.tile([P, 2 * W], BF16, tag="pT")
            nc.scalar.activation(
                out=pT, in_=st, func=mybir.ActivationFunctionType.Exp, scale=SCALE
            )
            pTs[j] = pT
            if j >= 1:  # mm2 for previous k-tile pair
                emit_mm2(j - 1, False)
            if j == 1 and pending_epilogue is not None:
                emit_epilogue_rest(pending_epilogue)
                pending_epilogue = None
            if j == 2 and w == 0 and h + 1 < H:
                heads[h + 1] = emit_head_load(h + 1)
            if j == 2 and w == NW - 1 and h + 1 < H:
                headsT[h + 1] = emit_head_transpose(*heads[h + 1][:2])
        emit_mm2(T // 2 - 1, True)
        # drain accumulator now (frees the single PSUM acc slot); rest deferred
        accs = accs_pool.tile([D + 1, W], F32, tag="accs")
        nc.vector.tensor_copy(accs, acc)
        pending_epilogue = (h, q0, accs)
    emit_epilogue_rest(pending_epilogue)


def build_nc():
    nc = bacc.Bacc("TRN2", target_bir_lowering=False, debug=False, num_devices=N_CORES)
    q = nc.declare_dram_parameter("q", [H, S, D], F32, isOutput=False)
    k = nc.declare_dram_parameter("k", [H, S, D], F32, isOutput=False)
    v = nc.declare_dram_parameter("v", [H, S, D], F32, isOutput=False)
    mask = nc.declare_dram_parameter("mask", [S], I32, isOutput=False)
    out = nc.declare_dram_parameter("out", [H, S, D], F32, isOutput=True)
    from contextlib import ExitStack

    with tile.TileContext(nc) as tc, ExitStack() as ctx:
        emit_core_program(ctx, nc, tc, q.ap(), k.ap(), v.ap(), mask.ap(), out.ap())
    nc.compile()
    return nc


_NC_CACHE = []


def get_nc():
    if not _NC_CACHE:
        _NC_CACHE.append(build_nc())
    return _NC_CACHE[0]


def make_in_maps(q, k, v, mask):
    """Shard full [B,NH,S,D] inputs into per-core input maps (8 heads/core)."""
    qf = np.asarray(q, dtype=np.float32).reshape(B * NH, S, D)
    kf = np.asarray(k, dtype=np.float32).reshape(B * NH, S, D)
    vf = np.asarray(v, dtype=np.float32).reshape(B * NH, S, D)
    mf = np.asarray(mask, dtype=np.int32).reshape(B, S)
    in_maps = []
    for c in range(N_CORES):
        lo = c * H
        in_maps.append(
            {
                "q": np.ascontiguousarray(qf[lo : lo + H]),
                "k": np.ascontiguousarray(kf[lo : lo + H]),
                "v": np.ascontiguousarray(vf[lo : lo + H]),
                # heads lo..lo+H-1 all belong to batch lo // NH
                "mask": np.ascontiguousarray(mf[lo // NH]),
            }
        )
    return in_maps


def kernel(q, k, v, mask):
    from concourse.bass_utils import run_bass_kernel_spmd

    nc = get_nc()
    in_maps = make_in_maps(q, k, v, mask)
    res = run_bass_kernel_spmd(nc, in_maps, list(range(N_CORES))).results
    out = np.concatenate([res[c]["out"] for c in range(N_CORES)], axis=0)
    return out.reshape(B, NH, S, D)


if __name__ == "__main__":
    nc = build_nc()
    print("built ok")


# revision 20
# speedup vs baseline: 1.9606x; 1.0482x over previous
"""Trainium2 Bass kernel for nn_BaseAttention (B=4, H=16, S=2048, D=64, key-mask).

Strategy (8 NeuronCores, batch*head sharded, 8 heads per core):
  For each head (Q,K,V: [S,D] f32, mask: [S] int 0/1):
    - Load Q,K,V with fp32->bf16 cast during DMA (SWDGE).
    - PE-transpose Q,K tiles -> Q^T, K^T [D=64, S] bf16 in SBUF.
    - Scores transposed: S^T[k, q] = K @ Q^T via matmul(lhsT=K^T chunk, rhs=Q^T),
      fp32 PSUM.  Softmax needs exp over k (partition axis in this layout), so
      mask+scale+exp fuse into ONE ScalarE pass:
          P^T = Exp(S^T * (1/sqrt(D)) + bias[k]),  bias[k] = -1e4 * mask[k]
      (per-partition bias).  No max-subtraction: scores are ~N(0,1), exp is safe
      in fp32, masked entries underflow to 0 exactly like the reference.
    - Denominator for free: V' = [V | ones] (M=64->65 doesn't change stream
      length), out'^T[0:64, q] = unnormalized out^T, out'^T[64, q] = sum(exp).
    - Reciprocal of the sums row, PE-transpose [65, q] -> [q, 65], multiply by
      per-partition recip, store.

Self-contained: hardcodes shapes; imports concourse from /opt/trn_rl_repo.
"""

import sys

if "/opt/trn_rl_repo" not in sys.path:
    sys.path.insert(0, "/opt/trn_rl_repo")

import numpy as np

import concourse.bass as bass
import concourse.mybir as mybir
import concourse.tile as tile
from concourse import bacc
from concourse.masks import make_identity

F32 = mybir.dt.float32
BF16 = mybir.dt.bfloat16
I32 = mybir.dt.int32

N_CORES = 8
B, NH, S, D = 4, 16, 2048, 64
H = (B * NH) // N_CORES  # heads per core = 8
P = 128                  # partitions / k-tile size
T = S // P               # 16 k-tiles per head
W = 512                  # q-window width (= matmul moving chunk, fp32 PSUM out limit)
NW = S // W              # 4 q-windows per head
SCALE = 1.0 / 8.0        # 1/sqrt(D)
NEG = -10000.0


def emit_core_program(ctx, nc, tc, q_h, k_h, v_h, mask_h, out_h):
    """Emit the per-core Tile program. q/k/v/out: DRAM APs [H, S, D]; mask: [S] i32."""
    pool = lambda *a, **kw: ctx.enter_context(tc.tile_pool(*a, **kw))
    singles = pool(name="singles", bufs=1)
    ld = pool(name="ld", bufs=2)            # SBUF head staging (bf16)
    qkT = pool(name="qkT", bufs=2)          # SBUF Q^T/K^T
    ppool = pool(name="p", bufs=5)          # SBUF P^T tiles (pair-lagged mm2)
    accs_pool = pool(name="accs", bufs=2)   # SBUF drained accumulators
    outs_pool = pool(name="outs", bufs=2)   # SBUF output staging
    st_pool = pool(name="st", bufs=2, space="PSUM")    # S^T pair tiles (2 banks ea)
    acc_pool = pool(name="acc", bufs=2, space="PSUM")  # out'^T accum (1 bank ea)
    tp_pool = pool(name="tp", bufs=2, space="PSUM")    # transposes (1 bank ea)

    ident_bf = singles.tile([P, P], BF16)
    make_identity(nc, ident_bf)
    ident_f32 = singles.tile([P, P], F32)
    make_identity(nc, ident_f32)

    # mask [S] i32 -> om [128, T] f32 = 1 - mask (om[p, t] = key t*128+p kept?)
    # The mask is applied by zeroing masked rows of V' (incl. the ones column):
    # out = sum_k exp(s_k) V'[k] makes that exactly equivalent to score masking.
    mask_i = singles.tile([P, T], I32)
    nc.sync.dma_start(out=mask_i, in_=mask_h.rearrange("(t p) -> p t", p=P))
    om = singles.tile([P, T], F32)
    nc.vector.tensor_scalar(om, mask_i, -1.0, 1.0, mybir.AluOpType.mult, mybir.AluOpType.add)

    # -------- software-pipelined emission --------
    # Per-engine streams are in-order, so emission order decides overlap:
    #  * mm2 lags mm1/exp by one k-tile (PE never waits on the exp it just fed)
    #  * window epilogue (transpose/recip/scale/store) is deferred into the
    #    next window's k-loop; the accumulator drain happens immediately so
    #    the single PSUM acc slot frees fast
    #  * next head's loads + Q/K transposes are emitted mid-window

    def emit_head_load(h):
        q_sb = ld.tile([P, T, D], BF16, tag="q_sb")
        nc.gpsimd.dma_start(out=q_sb, in_=q_h[h].rearrange("(t p) d -> p t d", p=P))
        k_sb = ld.tile([P, T, D], BF16, tag="k_sb")
        nc.gpsimd.dma_start(out=k_sb, in_=k_h[h].rearrange("(t p) d -> p t d", p=P))
        v_sb = ld.tile([P, T, D + 1], BF16, tag="v_sb")
        nc.gpsimd.dma_start(
            out=v_sb[:, :, 0:D], in_=v_h[h].rearrange("(t p) d -> p t d", p=P)
        )
        nc.vector.memset(v_sb[:, :, D : D + 1], 1.0)
        for t in range(T):  # zero masked key rows of V' (applies the mask)
            nc.vector.tensor_scalar_mul(v_sb[:, t, :], v_sb[:, t, :], om[:, t : t + 1])
        return q_sb, k_sb, v_sb

    def emit_head_transpose(q_sb, k_sb):
        # Q^T / K^T [64, S], then duplicate onto partitions 64-127 (SBUF->SBUF
        # DMA) so mm1 can run two k-tiles concurrently in the two PE row halves.
        qT = qkT.tile([2 * D, S], BF16, tag="qT")
        kT = qkT.tile([2 * D, S], BF16, tag="kT")
        for src, dst in ((q_sb, qT), (k_sb, kT)):
            for g in range(T // 4):
                tp = tp_pool.tile([D, 4 * P], BF16, tag="tp")
                for j in range(4):
                    nc.tensor.transpose(
                        tp[:, j * P : (j + 1) * P], src[:, 4 * g + j, :], ident_bf
                    )
                nc.vector.tensor_copy(dst[:D, 4 * g * P : 4 * (g + 1) * P], tp)
            nc.sync.dma_start(out=dst[D : 2 * D, :], in_=dst[:D, :])
        return qT, kT

    def emit_epilogue_rest(ep):
        # transpose [65, W] -> W/P tiles of [q=128, 65], normalize, store
        h, q0, accs = ep
        ost = outs_pool.tile([P, W // P, D], F32, tag="ost")
        for j in range(W // P):
            ot = tp_pool.tile([P, D + 1], F32, tag="tp")
            nc.tensor.transpose(
                ot, accs[:, j * P : (j + 1) * P], ident_f32[: D + 1, : D + 1]
            )
            nc.vector.reciprocal(ot[:, D : D + 1], ot[:, D : D + 1])
            nc.vector.tensor_scalar_mul(ost[:, j, :], ot[:, 0:D], ot[:, D : D + 1])
        nc.sync.dma_start(
            out=out_h[h, q0 : q0 + W, :].rearrange("(j p) d -> p j d", p=P),
            in_=ost,
        )

    # Flat pipeline over all (head, window, pair) units.  mm2 lags mm1/exp by
    # MM2_LAG units and epilogues lag one more, so every semaphore wait
    # reaching the in-order PE stream is already satisfied and the matmuls
    # chain back-to-back (drains hidden by the next fill).
    MM2_LAG = 2
    NP = T // 2  # k-tile pairs per window
    units = [(h, w, j) for h in range(H) for w in range(NW) for j in range(NP)]
    heads = {0: emit_head_load(0)}
    headsT = {0: emit_head_transpose(*heads[0][:2])}
    accs_by_window = {}
    pTs = {}
    pending_epi = []

    def emit_mm2(i):
        h, w, j = units[i]
        acc = accs_by_window[(h, w)]
        v_sb = heads[h][2]
        pT_prev = pTs.pop(i)
        for c, t in ((0, 2 * j), (1, 2 * j + 1)):
            nc.tensor.matmul(
                acc,
                lhsT=v_sb[:, t, :],
                rhs=pT_prev[:, c * W : (c + 1) * W],
                start=(j == 0 and c == 0),
                stop=(j == NP - 1 and c == 1),
            )
        if j == NP - 1:  # window done: drain accumulator, defer the rest
            accs = accs_pool.tile([D + 1, W], F32, tag="accs")
            nc.vector.tensor_copy(accs, acc)
            del accs_by_window[(h, w)]
            pending_epi.append((i + 1, (h, w * W, accs)))

    for i, (h, w, j) in enumerate(units):
        if w == 0 and j == 0 and h > 1:
            del heads[h - 2], headsT[h - 2]
        qT, kT = headsT[h]
        if j == 0:
            accs_by_window[(h, w)] = acc_pool.tile(
                [D + 1, W], F32, tag="acc", name=f"acc_{h}_{w}"
            )
        q0 = w * W
        # one PSUM tile holds S^T for both k-tiles of the pair side by side,
        # written by two concurrently-executing row-tiled matmuls
        st = st_pool.tile([P, 2 * W], F32, tag="st")
        for c, (t, lo) in enumerate(((2 * j, 0), (2 * j + 1, D))):
            nc.tensor.matmul(
                st[:, c * W : (c + 1) * W],
                lhsT=kT[lo : lo + D, t * P : (t + 1) * P],
                rhs=qT[lo : lo + D, q0 : q0 + W],
                start=True,
                stop=True,
            )
        pT = ppool.tile([P, 2 * W], BF16, tag="pT")
        nc.scalar.activation(
            out=pT, in_=st, func=mybir.ActivationFunctionType.Exp, scale=SCALE
        )
        pTs[i] = pT
        if i >= MM2_LAG:
            emit_mm2(i - MM2_LAG)
        while pending_epi and pending_epi[0][0] <= i - MM2_LAG:
            emit_epilogue_rest(pending_epi.pop(0)[1])
        if j == 2 and w == 0 and h + 1 < H:
            heads[h + 1] = emit_head_load(h + 1)
        if j == 2 and w == NW - 1 and h + 1 < H:
            headsT[h + 1] = emit_head_transpose(*heads[h + 1][:2])
    for i in range(len(units) - MM2_LAG, len(units)):
        emit_mm2(i)
    for _, ep in pending_epi:
        emit_epilogue_rest(ep)


def build_nc():
    nc = bacc.Bacc("TRN2", target_bir_lowering=False, debug=False, num_devices=N_CORES)
    q = nc.declare_dram_parameter("q", [H, S, D], F32, isOutput=False)
    k = nc.declare_dram_parameter("k", [H, S, D], F32, isOutput=False)
    v = nc.declare_dram_parameter("v", [H, S, D], F32, isOutput=False)
    mask = nc.declare_dram_parameter("mask", [S], I32, isOutput=False)
    out = nc.declare_dram_parameter("out", [H, S, D], F32, isOutput=True)
    from contextlib import ExitStack

    with tile.TileContext(nc) as tc, ExitStack() as ctx:
        emit_core_program(ctx, nc, tc, q.ap(), k.ap(), v.ap(), mask.ap(), out.ap())
    nc.compile()
    return nc


_NC_CACHE = []


def get_nc():
    if not _NC_CACHE:
        _NC_CACHE.append(build_nc())
    return _NC_CACHE[0]


def make_in_maps(q, k, v, mask):
    """Shard full [B,NH,S,D] inputs into per-core input maps (8 heads/core)."""
    qf = np.asarray(q, dtype=np.float32).reshape(B * NH, S, D)
    kf = np.asarray(k, dtype=np.float32).reshape(B * NH, S, D)
    vf = np.asarray(v, dtype=np.float32).reshape(B * NH, S, D)
    mf = np.asarray(mask, dtype=np.int32).reshape(B, S)
    in_maps = []
    for c in range(N_CORES):
        lo = c * H
        in_maps.append(
            {
                "q": np.ascontiguousarray(qf[lo : lo + H]),
                "k": np.ascontiguousarray(kf[lo : lo + H]),
                "v": np.ascontiguousarray(vf[lo : lo + H]),
                # heads lo..lo+H-1 all belong to batch lo // NH
                "mask": np.ascontiguousarray(mf[lo // NH]),
            }
        )
    return in_maps


def kernel(q, k, v, mask):
    from concourse.bass_utils import run_bass_kernel_spmd

    nc = get_nc()
    in_maps = make_in_maps(q, k, v, mask)
    res = run_bass_kernel_spmd(nc, in_maps, list(range(N_CORES))).results
    out = np.concatenate([res[c]["out"] for c in range(N_CORES)], axis=0)
    return out.reshape(B, NH, S, D)


if __name__ == "__main__":
    nc = build_nc()
    print("built ok")


# revision 28
# speedup vs baseline: 2.1381x; 1.0905x over previous
"""Trainium2 Bass kernel for nn_BaseAttention (B=4, H=16, S=2048, D=64, key-mask).

Strategy (8 NeuronCores, batch*head sharded, 8 heads per core):
  For each head (Q,K,V: [S,D] f32, mask: [S] int 0/1):
    - Load Q,K,V with fp32->bf16 cast during DMA (SWDGE).
    - PE-transpose Q,K tiles -> Q^T, K^T [D=64, S] bf16 in SBUF.
    - Scores transposed: S^T[k, q] = K @ Q^T via matmul(lhsT=K^T chunk, rhs=Q^T),
      fp32 PSUM.  Softmax needs exp over k (partition axis in this layout), so
      mask+scale+exp fuse into ONE ScalarE pass:
          P^T = Exp(S^T * (1/sqrt(D)) + bias[k]),  bias[k] = -1e4 * mask[k]
      (per-partition bias).  No max-subtraction: scores are ~N(0,1), exp is safe
      in fp32, masked entries underflow to 0 exactly like the reference.
    - Denominator for free: V' = [V | ones] (M=64->65 doesn't change stream
      length), out'^T[0:64, q] = unnormalized out^T, out'^T[64, q] = sum(exp).
    - Reciprocal of the sums row, PE-transpose [65, q] -> [q, 65], multiply by
      per-partition recip, store.

Self-contained: hardcodes shapes; imports concourse from /opt/trn_rl_repo.
"""

import sys

if "/opt/trn_rl_repo" not in sys.path:
    sys.path.insert(0, "/opt/trn_rl_repo")

import numpy as np

import concourse.bass as bass
import concourse.mybir as mybir
import concourse.tile as tile
from concourse import bacc
from concourse.masks import make_identity

F32 = mybir.dt.float32
BF16 = mybir.dt.bfloat16
I32 = mybir.dt.int32

N_CORES = 8
B, NH, S, D = 4, 16, 2048, 64
H = (B * NH) // N_CORES  # heads per core = 8
P = 128                  # partitions / k-tile size
T = S // P               # 16 k-tiles per head
W = 512                  # q-window width (= matmul moving chunk, fp32 PSUM out limit)
NW = S // W              # 4 q-windows per head
SCALE = 1.0 / 8.0        # 1/sqrt(D)
NEG = -10000.0


def emit_core_program(ctx, nc, tc, q_h, k_h, v_h, mask_h, out_h):
    """Emit the per-core Tile program. q/k/v/out: DRAM APs [H, S, D]; mask: [S] i32."""
    pool = lambda *a, **kw: ctx.enter_context(tc.tile_pool(*a, **kw))
    singles = pool(name="singles", bufs=1)
    ld = pool(name="ld", bufs=2)            # SBUF head staging (bf16)
    qkT = pool(name="qkT", bufs=2)          # SBUF Q^T/K^T
    ppool = pool(name="p", bufs=5)          # SBUF P^T tiles (pair-lagged mm2)
    accs_pool = pool(name="accs", bufs=2)   # SBUF drained accumulators
    outs_pool = pool(name="outs", bufs=2)   # SBUF output staging
    st_pool = pool(name="st", bufs=2, space="PSUM")    # S^T pair tiles (2 banks ea)
    acc_pool = pool(name="acc", bufs=2, space="PSUM")  # out'^T accum (1 bank ea)
    tp_pool = pool(name="tp", bufs=2, space="PSUM")    # transposes (1 bank ea)

    ident_bf = singles.tile([P, P], BF16)
    make_identity(nc, ident_bf)
    ident_f32 = singles.tile([P, P], F32)
    make_identity(nc, ident_f32)

    # mask [S] i32 -> om [128, T] f32 = 1 - mask (om[p, t] = key t*128+p kept?)
    # The mask is applied by zeroing masked rows of V' (incl. the ones column):
    # out = sum_k exp(s_k) V'[k] makes that exactly equivalent to score masking.
    mask_i = singles.tile([P, T], I32)
    nc.sync.dma_start(out=mask_i, in_=mask_h.rearrange("(t p) -> p t", p=P))
    om = singles.tile([P, T], F32)
    nc.vector.tensor_scalar(om, mask_i, -1.0, 1.0, mybir.AluOpType.mult, mybir.AluOpType.add)

    # -------- software-pipelined emission --------
    # Per-engine streams are in-order, so emission order decides overlap:
    #  * mm2 lags mm1/exp by one k-tile (PE never waits on the exp it just fed)
    #  * window epilogue (transpose/recip/scale/store) is deferred into the
    #    next window's k-loop; the accumulator drain happens immediately so
    #    the single PSUM acc slot frees fast
    #  * next head's loads + Q/K transposes are emitted mid-window

    def emit_head_load(h):
        q_sb = ld.tile([P, T, D], BF16, tag="q_sb")
        nc.gpsimd.dma_start(out=q_sb, in_=q_h[h].rearrange("(t p) d -> p t d", p=P))
        k_sb = ld.tile([P, T, D], BF16, tag="k_sb")
        nc.gpsimd.dma_start(out=k_sb, in_=k_h[h].rearrange("(t p) d -> p t d", p=P))
        v_sb = ld.tile([P, T, D + 1], BF16, tag="v_sb")
        nc.gpsimd.dma_start(
            out=v_sb[:, :, 0:D], in_=v_h[h].rearrange("(t p) d -> p t d", p=P)
        )
        nc.vector.memset(v_sb[:, :, D : D + 1], 1.0)
        for t in range(T):  # zero masked key rows of V' (applies the mask)
            nc.vector.tensor_scalar_mul(v_sb[:, t, :], v_sb[:, t, :], om[:, t : t + 1])
        return q_sb, k_sb, v_sb

    def head_prep_thunks(h):
        # Q^T / K^T [64, S], then duplicate onto partitions 64-127 (SBUF->SBUF
        # DMA) so mm1 can run two k-tiles concurrently in the two PE row halves.
        # Split into small thunks so the PE work spreads across many units.
        q_sb, k_sb, _ = heads[h]

        def alloc():
            qT = qkT.tile([2 * D, S], BF16, tag="qT", name=f"qT_{h}")
            kT = qkT.tile([2 * D, S], BF16, tag="kT", name=f"kT_{h}")
            headsT[h] = (qT, kT)

        def group(which, g):
            def f():
                src = q_sb if which == 0 else k_sb
                dst = headsT[h][which]
                tp = tp_pool.tile(
                    [D, 4 * P], BF16, tag="tp", name=f"tp_{h}_{which}_{g}"
                )
                for jj in range(4):
                    nc.tensor.transpose(
                        tp[:, jj * P : (jj + 1) * P], src[:, 4 * g + jj, :], ident_bf
                    )
                nc.vector.tensor_copy(dst[:D, 4 * g * P : 4 * (g + 1) * P], tp)

            return f

        def dup(which):
            def f():
                dst = headsT[h][which]
                nc.sync.dma_start(out=dst[D : 2 * D, :], in_=dst[:D, :])

            return f

        first = group(0, 0)
        thunks = [lambda: (alloc(), first())]
        thunks += [group(0, g) for g in range(1, T // 4)]
        thunks += [lambda: (dup(0)(), group(1, 0)())]
        thunks += [group(1, g) for g in range(1, T // 4)]
        thunks.append(dup(1))
        return thunks

    def emit_head_transpose_now(h):
        for t in head_prep_thunks(h):
            t()
        return headsT[h]

    F32R = mybir.dt.float32r

    def emit_epilogue_rest(ep):
        # transpose [65, W] -> W/P tiles of [q=128, 65], normalize, store.
        # float32r views: single-pass PE transpose (fp32 is two-pass), tf32-ish
        # rounding of already-rounded values is negligible here.
        h, q0, accs = ep
        ost = outs_pool.tile([P, W // P, D], F32, tag="ost")
        for j in range(W // P):
            ot = tp_pool.tile([P, D + 1], F32, tag="tp")
            nc.tensor.transpose(
                ot, accs[:, j * P : (j + 1) * P], ident_f32[: D + 1, : D + 1]
            )
            nc.vector.reciprocal(ot[:, D : D + 1], ot[:, D : D + 1])
            nc.vector.tensor_scalar_mul(ost[:, j, :], ot[:, 0:D], ot[:, D : D + 1])
        nc.sync.dma_start(
            out=out_h[h, q0 : q0 + W, :].rearrange("(j p) d -> p j d", p=P),
            in_=ost,
        )

    # Flat pipeline over all (head, window, pair) units.  mm2 lags mm1/exp by
    # MM2_LAG units and epilogues lag one more, so every semaphore wait
    # reaching the in-order PE stream is already satisfied and the matmuls
    # chain back-to-back (drains hidden by the next fill).
    MM2_LAG = 2
    NP = T // 2  # k-tile pairs per window
    units = [(h, w, j) for h in range(H) for w in range(NW) for j in range(NP)]
    heads = {0: emit_head_load(0)}
    headsT = {}
    accs_by_window = {}
    pTs = {}
    pending_epi = []
    work_queue = []
    emit_head_transpose_now(0)

    def emit_mm2(i):
        h, w, j = units[i]
        acc = accs_by_window[(h, w)]
        v_sb = heads[h][2]
        pT_prev = pTs.pop(i)
        for c, t in ((0, 2 * j), (1, 2 * j + 1)):
            nc.tensor.matmul(
                acc,
                lhsT=v_sb[:, t, :],
                rhs=pT_prev[:, c * W : (c + 1) * W],
                start=(j == 0 and c == 0),
                stop=(j == NP - 1 and c == 1),
            )
        if j == NP - 1:  # window done: drain accumulator, defer the rest
            accs = accs_pool.tile([D + 1, W], F32, tag="accs")
            nc.vector.tensor_copy(accs, acc)
            del accs_by_window[(h, w)]
            pending_epi.append((i + 1, (h, w * W, accs)))

    for i, (h, w, j) in enumerate(units):
        if w == 0 and j == 0 and h > 1:
            del heads[h - 2], headsT[h - 2]
        qT, kT = headsT[h]
        if j == 0:
            accs_by_window[(h, w)] = acc_pool.tile(
                [D + 1, W], F32, tag="acc", name=f"acc_{h}_{w}"
            )
        q0 = w * W
        # one PSUM tile holds S^T for both k-tiles of the pair side by side,
        # written by two concurrently-executing row-tiled matmuls
        st = st_pool.tile([P, 2 * W], F32, tag="st")
        for c, (t, lo) in enumerate(((2 * j, 0), (2 * j + 1, D))):
            nc.tensor.matmul(
                st[:, c * W : (c + 1) * W],
                lhsT=kT[lo : lo + D, t * P : (t + 1) * P],
                rhs=qT[lo : lo + D, q0 : q0 + W],
                start=True,
                stop=True,
            )
        pT = ppool.tile([P, 2 * W], BF16, tag="pT")
        nc.scalar.activation(
            out=pT, in_=st, func=mybir.ActivationFunctionType.Exp, scale=SCALE
        )
        pTs[i] = pT
        if i >= MM2_LAG:
            emit_mm2(i - MM2_LAG)
        while pending_epi and pending_epi[0][0] <= i - MM2_LAG:
            emit_epilogue_rest(pending_epi.pop(0)[1])
        if j == 2 and w == 0 and h + 1 < H:
            heads[h + 1] = emit_head_load(h + 1)
        if j == 0 and w == 2 and h + 1 < H:
            work_queue.extend(head_prep_thunks(h + 1))
        if work_queue:
            work_queue.pop(0)()
    for i in range(len(units) - MM2_LAG, len(units)):
        emit_mm2(i)
    for _, ep in pending_epi:
        emit_epilogue_rest(ep)


def build_nc():
    nc = bacc.Bacc("TRN2", target_bir_lowering=False, debug=False, num_devices=N_CORES)
    q = nc.declare_dram_parameter("q", [H, S, D], F32, isOutput=False)
    k = nc.declare_dram_parameter("k", [H, S, D], F32, isOutput=False)
    v = nc.declare_dram_parameter("v", [H, S, D], F32, isOutput=False)
    mask = nc.declare_dram_parameter("mask", [S], I32, isOutput=False)
    out = nc.declare_dram_parameter("out", [H, S, D], F32, isOutput=True)
    from contextlib import ExitStack

    with tile.TileContext(nc) as tc, ExitStack() as ctx:
        emit_core_program(ctx, nc, tc, q.ap(), k.ap(), v.ap(), mask.ap(), out.ap())
    nc.compile()
    return nc


_NC_CACHE = []


def get_nc():
    if not _NC_CACHE:
        _NC_CACHE.append(build_nc())
    return _NC_CACHE[0]


def make_in_maps(q, k, v, mask):
    """Shard full [B,NH,S,D] inputs into per-core input maps (8 heads/core)."""
    qf = np.asarray(q, dtype=np.float32).reshape(B * NH, S, D)
    kf = np.asarray(k, dtype=np.float32).reshape(B * NH, S, D)
    vf = np.asarray(v, dtype=np.float32).reshape(B * NH, S, D)
    mf = np.asarray(mask, dtype=np.int32).reshape(B, S)
    in_maps = []
    for c in range(N_CORES):
        lo = c * H
        in_maps.append(
            {
                "q": np.ascontiguousarray(qf[lo : lo + H]),
                "k": np.ascontiguousarray(kf[lo : lo + H]),
                "v": np.ascontiguousarray(vf[lo : lo + H]),
                # heads lo..lo+H-1 all belong to batch lo // NH
                "mask": np.ascontiguousarray(mf[lo // NH]),
            }
        )
    return in_maps


def kernel(q, k, v, mask):
    from concourse.bass_utils import run_bass_kernel_spmd

    nc = get_nc()
    in_maps = make_in_maps(q, k, v, mask)
    res = run_bass_kernel_spmd(nc, in_maps, list(range(N_CORES))).results
    out = np.concatenate([res[c]["out"] for c in range(N_CORES)], axis=0)
    return out.reshape(B, NH, S, D)


if __name__ == "__main__":
    nc = build_nc()
    print("built ok")


# revision 30
# speedup vs baseline: 2.1806x; 1.0199x over previous
"""Trainium2 Bass kernel for nn_BaseAttention (B=4, H=16, S=2048, D=64, key-mask).

Strategy (8 NeuronCores, batch*head sharded, 8 heads per core):
  For each head (Q,K,V: [S,D] f32, mask: [S] int 0/1):
    - Load Q,K,V with fp32->bf16 cast during DMA (SWDGE).
    - PE-transpose Q,K tiles -> Q^T, K^T [D=64, S] bf16 in SBUF.
    - Scores transposed: S^T[k, q] = K @ Q^T via matmul(lhsT=K^T chunk, rhs=Q^T),
      fp32 PSUM.  Softmax needs exp over k (partition axis in this layout), so
      mask+scale+exp fuse into ONE ScalarE pass:
          P^T = Exp(S^T * (1/sqrt(D)) + bias[k]),  bias[k] = -1e4 * mask[k]
      (per-partition bias).  No max-subtraction: scores are ~N(0,1), exp is safe
      in fp32, masked entries underflow to 0 exactly like the reference.
    - Denominator for free: V' = [V | ones] (M=64->65 doesn't change stream
      length), out'^T[0:64, q] = unnormalized out^T, out'^T[64, q] = sum(exp).
    - Reciprocal of the sums row, PE-transpose [65, q] -> [q, 65], multiply by
      per-partition recip, store.

Self-contained: hardcodes shapes; imports concourse from /opt/trn_rl_repo.
"""

import sys

if "/opt/trn_rl_repo" not in sys.path:
    sys.path.insert(0, "/opt/trn_rl_repo")

import numpy as np

import concourse.bass as bass
import concourse.mybir as mybir
import concourse.tile as tile
from concourse import bacc
from concourse.masks import make_identity

F32 = mybir.dt.float32
BF16 = mybir.dt.bfloat16
I32 = mybir.dt.int32

N_CORES = 8
B, NH, S, D = 4, 16, 2048, 64
H = (B * NH) // N_CORES  # heads per core = 8
P = 128                  # partitions / k-tile size
T = S // P               # 16 k-tiles per head
W = 512                  # q-window width (= matmul moving chunk, fp32 PSUM out limit)
NW = S // W              # 4 q-windows per head
SCALE = 1.0 / 8.0        # 1/sqrt(D)
NEG = -10000.0


def emit_core_program(ctx, nc, tc, q_h, k_h, v_h, mask_h, out_h):
    """Emit the per-core Tile program. q/k/v/out: DRAM APs [H, S, D]; mask: [S] i32."""
    pool = lambda *a, **kw: ctx.enter_context(tc.tile_pool(*a, **kw))
    singles = pool(name="singles", bufs=1)
    ld = pool(name="ld", bufs=2)            # SBUF head staging (bf16)
    qkT = pool(name="qkT", bufs=2)          # SBUF Q^T/K^T
    ppool = pool(name="p", bufs=5)          # SBUF P^T tiles (pair-lagged mm2)
    accs_pool = pool(name="accs", bufs=2)   # SBUF drained accumulators
    outs_pool = pool(name="outs", bufs=2)   # SBUF output staging
    st_pool = pool(name="st", bufs=2, space="PSUM")    # S^T pair tiles (2 banks ea)
    acc_pool = pool(name="acc", bufs=2, space="PSUM")  # out'^T accum (1 bank ea)
    tp_pool = pool(name="tp", bufs=2, space="PSUM")    # transposes (1 bank ea)

    ident_bf = singles.tile([P, P], BF16)
    make_identity(nc, ident_bf)
    ident_f32 = singles.tile([P, P], F32)
    make_identity(nc, ident_f32)

    # mask [S] i32 -> om [128, T] f32 = 1 - mask (om[p, t] = key t*128+p kept?)
    # The mask is applied by zeroing masked rows of V' (incl. the ones column):
    # out = sum_k exp(s_k) V'[k] makes that exactly equivalent to score masking.
    mask_i = singles.tile([P, T], I32)
    nc.sync.dma_start(out=mask_i, in_=mask_h.rearrange("(t p) -> p t", p=P))
    om = singles.tile([P, T], F32)
    nc.vector.tensor_scalar(om, mask_i, -1.0, 1.0, mybir.AluOpType.mult, mybir.AluOpType.add)

    # -------- software-pipelined emission --------
    # Per-engine streams are in-order, so emission order decides overlap:
    #  * mm2 lags mm1/exp by one k-tile (PE never waits on the exp it just fed)
    #  * window epilogue (transpose/recip/scale/store) is deferred into the
    #    next window's k-loop; the accumulator drain happens immediately so
    #    the single PSUM acc slot frees fast
    #  * next head's loads + Q/K transposes are emitted mid-window

    def emit_head_load(h):
        q_sb = ld.tile([P, T, D], BF16, tag="q_sb")
        nc.gpsimd.dma_start(out=q_sb, in_=q_h[h].rearrange("(t p) d -> p t d", p=P))
        k_sb = ld.tile([P, T, D], BF16, tag="k_sb")
        nc.gpsimd.dma_start(out=k_sb, in_=k_h[h].rearrange("(t p) d -> p t d", p=P))
        v_sb = ld.tile([P, T, D + 1], BF16, tag="v_sb")
        nc.gpsimd.dma_start(
            out=v_sb[:, :, 0:D], in_=v_h[h].rearrange("(t p) d -> p t d", p=P)
        )
        nc.vector.memset(v_sb[:, :, D : D + 1], 1.0)
        for t in range(T):  # zero masked key rows of V' (applies the mask)
            nc.vector.tensor_scalar_mul(v_sb[:, t, :], v_sb[:, t, :], om[:, t : t + 1])
        return q_sb, k_sb, v_sb

    def head_prep_thunks(h):
        # Q^T / K^T [64, S], then duplicate onto partitions 64-127 (SBUF->SBUF
        # DMA) so mm1 can run two k-tiles concurrently in the two PE row halves.
        # Split into small thunks so the PE work spreads across many units.
        q_sb, k_sb, _ = heads[h]

        def alloc():
            qT = qkT.tile([2 * D, S], BF16, tag="qT", name=f"qT_{h}")
            kT = qkT.tile([2 * D, S], BF16, tag="kT", name=f"kT_{h}")
            headsT[h] = (qT, kT)

        def group(which, g):
            def f():
                src = q_sb if which == 0 else k_sb
                dst = headsT[h][which]
                cols = slice(4 * g * P, 4 * (g + 1) * P)
                tp = tp_pool.tile(
                    [D, 4 * P], BF16, tag="tp", name=f"tp_{h}_{which}_{g}"
                )
                for jj in range(4):
                    nc.tensor.transpose(
                        tp[:, jj * P : (jj + 1) * P], src[:, 4 * g + jj, :], ident_bf
                    )
                nc.vector.tensor_copy(dst[:D, cols], tp)
                # duplicate this slice onto partitions 64-127 right away
                nc.sync.dma_start(out=dst[D : 2 * D, cols], in_=dst[:D, cols])

            return f

        first = group(1, 0)
        thunks = [lambda: (alloc(), first())]
        thunks += [group(0, 0)]
        for g in range(1, T // 4):  # interleave K and Q groups
            thunks += [group(1, g), group(0, g)]
        return thunks

    def emit_head_transpose_now(h):
        for t in head_prep_thunks(h):
            t()
        return headsT[h]

    F32R = mybir.dt.float32r

    def emit_epilogue_rest(ep):
        # transpose [65, W] -> W/P tiles of [q=128, 65], normalize, store.
        # float32r views: single-pass PE transpose (fp32 is two-pass), tf32-ish
        # rounding of already-rounded values is negligible here.
        h, q0, accs = ep
        ost = outs_pool.tile([P, W // P, D], F32, tag="ost")
        for j in range(W // P):
            ot = tp_pool.tile([P, D + 1], F32, tag="tp")
            nc.tensor.transpose(
                ot, accs[:, j * P : (j + 1) * P], ident_f32[: D + 1, : D + 1]
            )
            nc.vector.reciprocal(ot[:, D : D + 1], ot[:, D : D + 1])
            nc.vector.tensor_scalar_mul(ost[:, j, :], ot[:, 0:D], ot[:, D : D + 1])
        nc.sync.dma_start(
            out=out_h[h, q0 : q0 + W, :].rearrange("(j p) d -> p j d", p=P),
            in_=ost,
        )

    # Flat pipeline over all (head, window, pair) units.  mm2 lags mm1/exp by
    # MM2_LAG units and epilogues lag one more, so every semaphore wait
    # reaching the in-order PE stream is already satisfied and the matmuls
    # chain back-to-back (drains hidden by the next fill).
    MM2_LAG = 2
    NP = T // 2  # k-tile pairs per window
    units = [(h, w, j) for h in range(H) for w in range(NW) for j in range(NP)]
    heads = {0: emit_head_load(0)}
    headsT = {}
    accs_by_window = {}
    pTs = {}
    pending_epi = []
    work_queue = []
    emit_head_transpose_now(0)

    def emit_mm2(i):
        h, w, j = units[i]
        acc = accs_by_window[(h, w)]
        v_sb = heads[h][2]
        pT_prev = pTs.pop(i)
        for c, t in ((0, 2 * j), (1, 2 * j + 1)):
            nc.tensor.matmul(
                acc,
                lhsT=v_sb[:, t, :],
                rhs=pT_prev[:, c * W : (c + 1) * W],
                start=(j == 0 and c == 0),
                stop=(j == NP - 1 and c == 1),
            )
        if j == NP - 1:  # window done: drain accumulator, defer the rest
            accs = accs_pool.tile([D + 1, W], F32, tag="accs")
            nc.vector.tensor_copy(accs, acc)
            del accs_by_window[(h, w)]
            pending_epi.append((i + 1, (h, w * W, accs)))

    for i, (h, w, j) in enumerate(units):
        if w == 0 and j == 0 and h > 1:
            del heads[h - 2], headsT[h - 2]
        qT, kT = headsT[h]
        if j == 0:
            accs_by_window[(h, w)] = acc_pool.tile(
                [D + 1, W], F32, tag="acc", name=f"acc_{h}_{w}"
            )
        q0 = w * W
        # one PSUM tile holds S^T for both k-tiles of the pair side by side,
        # written by two concurrently-executing row-tiled matmuls
        st = st_pool.tile([P, 2 * W], F32, tag="st")
        for c, (t, lo) in enumerate(((2 * j, 0), (2 * j + 1, D))):
            nc.tensor.matmul(
                st[:, c * W : (c + 1) * W],
                lhsT=kT[lo : lo + D, t * P : (t + 1) * P],
                rhs=qT[lo : lo + D, q0 : q0 + W],
                start=True,
                stop=True,
            )
        pT = ppool.tile([P, 2 * W], BF16, tag="pT")
        nc.scalar.activation(
            out=pT, in_=st, func=mybir.ActivationFunctionType.Exp, scale=SCALE
        )
        pTs[i] = pT
        if i >= MM2_LAG:
            emit_mm2(i - MM2_LAG)
        while pending_epi and pending_epi[0][0] <= i - MM2_LAG:
            emit_epilogue_rest(pending_epi.pop(0)[1])
        if j == 2 and w == 0 and h + 1 < H:
            heads[h + 1] = emit_head_load(h + 1)
        if j == 0 and w == 1 and h + 1 < H:
            work_queue.extend(head_prep_thunks(h + 1))
        if work_queue:
            work_queue.pop(0)()
    for i in range(len(units) - MM2_LAG, len(units)):
        emit_mm2(i)
    for _, ep in pending_epi:
        emit_epilogue_rest(ep)


def build_nc():
    nc = bacc.Bacc("TRN2", target_bir_lowering=False, debug=False, num_devices=N_CORES)
    q = nc.declare_dram_parameter("q", [H, S, D], F32, isOutput=False)
    k = nc.declare_dram_parameter("k", [H, S, D], F32, isOutput=False)
    v = nc.declare_dram_parameter("v", [H, S, D], F32, isOutput=False)
    mask = nc.declare_dram_parameter("mask", [S], I32, isOutput=False)
    out = nc.declare_dram_parameter("out", [H, S, D], F32, isOutput=True)
    from contextlib import ExitStack

    with tile.TileContext(nc) as tc, ExitStack() as ctx:
        emit_core_program(ctx, nc, tc, q.ap(), k.ap(), v.ap(), mask.ap(), out.ap())
    nc.compile()
    return nc


_NC_CACHE = []


def get_nc():
    if not _NC_CACHE:
        _NC_CACHE.append(build_nc())
    return _NC_CACHE[0]


def make_in_maps(q, k, v, mask):
    """Shard full [B,NH,S,D] inputs into per-core input maps (8 heads/core)."""
    qf = np.asarray(q, dtype=np.float32).reshape(B * NH, S, D)
    kf = np.asarray(k, dtype=np.float32).reshape(B * NH, S, D)
    vf = np.asarray(v, dtype=np.float32).reshape(B * NH, S, D)
    mf = np.asarray(mask, dtype=np.int32).reshape(B, S)
    in_maps = []
    for c in range(N_CORES):
        lo = c * H
        in_maps.append(
            {
                "q": np.ascontiguousarray(qf[lo : lo + H]),
                "k": np.ascontiguousarray(kf[lo : lo + H]),
                "v": np.ascontiguousarray(vf[lo : lo + H]),
                # heads lo..lo+H-1 all belong to batch lo // NH
                "mask": np.ascontiguousarray(mf[lo // NH]),
            }
        )
    return in_maps


def kernel(q, k, v, mask):
    from concourse.bass_utils import run_bass_kernel_spmd

    nc = get_nc()
    in_maps = make_in_maps(q, k, v, mask)
    res = run_bass_kernel_spmd(nc, in_maps, list(range(N_CORES))).results
    out = np.concatenate([res[c]["out"] for c in range(N_CORES)], axis=0)
    return out.reshape(B, NH, S, D)


if __name__ == "__main__":
    nc = build_nc()
    print("built ok")


# revision 33
# speedup vs baseline: 2.1860x; 1.0025x over previous
"""Trainium2 Bass kernel for nn_BaseAttention (B=4, H=16, S=2048, D=64, key-mask).

Strategy (8 NeuronCores, batch*head sharded, 8 heads per core; each core's 8
heads happen to share one batch's mask):
  For each head (Q,K,V: [S,D] f32, mask: [S] int 0/1):
    - Load Q,K,V with fp32->bf16 cast during DMA (SWDGE).
    - PE-transpose Q,K tiles -> Q^T, K^T [64, S] bf16, duplicated onto
      partitions 64-127 so mm1 can run two k-tiles concurrently in the two
      row halves of the PE array (row tiling, K=64 each).
    - Scores transposed: S^T[k, q] = K @ Q^T, fp32 PSUM, one [128, 2*512]
      tile per k-tile pair; one ScalarE pass computes P^T = Exp(S^T/8).
      No max-subtraction: scores are ~N(0,1) so exp cannot overflow, and no
      additive mask: the key mask is applied by zeroing masked rows of
      V' = [V | ones] (out = sum_k exp(s_k) V'[k] makes that exactly
      equivalent, including the softmax denominator in the ones column).
    - mm2 accumulates out'^T [65, q] over k; the sums row is the denominator.
    - Reciprocal of sums, PE-transpose [65, q] -> [q, 65], scale, store.
  Emission is a flat software pipeline over (head, window, k-pair) units with
  mm2 and epilogues lagging 2 units, so the in-order PE stream never reaches
  an unmet semaphore and matmuls chain back-to-back.

Self-contained: hardcodes shapes; imports concourse from /opt/trn_rl_repo.
"""

import sys

if "/opt/trn_rl_repo" not in sys.path:
    sys.path.insert(0, "/opt/trn_rl_repo")

import numpy as np

import concourse.bass as bass
import concourse.mybir as mybir
import concourse.tile as tile
from concourse import bacc
from concourse.masks import make_identity

F32 = mybir.dt.float32
BF16 = mybir.dt.bfloat16
I32 = mybir.dt.int32

N_CORES = 8
B, NH, S, D = 4, 16, 2048, 64
H = (B * NH) // N_CORES  # heads per core = 8
P = 128                  # partitions / k-tile size
T = S // P               # 16 k-tiles per head
W = 512                  # q-window width (= fp32 PSUM bank limit per matmul)
NW = S // W              # 4 q-windows per head
SCALE = 1.0 / 8.0        # 1/sqrt(D)


def emit_core_program(ctx, nc, tc, q_h, k_h, v_h, mask_h, out_h):
    """Emit the per-core Tile program. q/k/v/out: DRAM APs [H, S, D]; mask: [S] i32."""
    pool = lambda *a, **kw: ctx.enter_context(tc.tile_pool(*a, **kw))
    singles = pool(name="singles", bufs=1)
    ld = pool(name="ld", bufs=2)            # SBUF head staging (bf16)
    qkT = pool(name="qkT", bufs=2)          # SBUF Q^T/K^T (both row halves)
    ppool = pool(name="p", bufs=5)          # SBUF P^T tiles (lagged mm2)
    accs_pool = pool(name="accs", bufs=2)   # SBUF drained accumulators
    outs_pool = pool(name="outs", bufs=2)   # SBUF output staging
    st_pool = pool(name="st", bufs=2, space="PSUM")    # S^T pair tiles (2 banks ea)
    acc_pool = pool(name="acc", bufs=2, space="PSUM")  # out'^T accum (1 bank ea)
    tp_pool = pool(name="tp", bufs=2, space="PSUM")    # transposes (1 bank ea)

    ident_bf = singles.tile([P, P], BF16)
    make_identity(nc, ident_bf)
    ident_f32 = singles.tile([P, P], F32)
    make_identity(nc, ident_f32)

    # mask [S] i32 -> om [128, T] f32 = 1 - mask  (om[p, t] = keep key t*128+p)
    mask_i = singles.tile([P, T], I32)
    nc.sync.dma_start(out=mask_i, in_=mask_h.rearrange("(t p) -> p t", p=P))
    om = singles.tile([P, T], F32)
    nc.vector.tensor_scalar(
        om, mask_i, -1.0, 1.0, mybir.AluOpType.mult, mybir.AluOpType.add
    )

    def emit_head_load(h):
        q_sb = ld.tile([P, T, D], BF16, tag="q_sb", name=f"q_sb_{h}")
        nc.gpsimd.dma_start(out=q_sb, in_=q_h[h].rearrange("(t p) d -> p t d", p=P))
        k_sb = ld.tile([P, T, D], BF16, tag="k_sb", name=f"k_sb_{h}")
        nc.gpsimd.dma_start(out=k_sb, in_=k_h[h].rearrange("(t p) d -> p t d", p=P))
        v_sb = ld.tile([P, T, D + 1], BF16, tag="v_sb", name=f"v_sb_{h}")
        nc.gpsimd.dma_start(
            out=v_sb[:, :, 0:D], in_=v_h[h].rearrange("(t p) d -> p t d", p=P)
        )
        nc.vector.memset(v_sb[:, :, D : D + 1], 1.0)
        # zero masked key rows of V' (applies the mask): one multiply with
        # om broadcast along d via a zero-stride AP dim
        om_b = bass.AP(tensor=om.tensor, offset=om.offset, ap=om.ap + [[0, D + 1]])
        nc.vector.tensor_mul(v_sb, v_sb, om_b)
        return q_sb, k_sb, v_sb

    def head_prep_thunks(h):
        # Q^T / K^T [64, S] bf16, each 512-col slice duplicated onto
        # partitions 64-127 right after it is built (SBUF->SBUF DMA) so mm1
        # row-tile pairs never wait long on a duplicate.  Split into small
        # thunks so the PE work spreads across many pipeline units.
        q_sb, k_sb, _ = heads[h]

        def alloc():
            qT = qkT.tile([2 * D, S], BF16, tag="qT", name=f"qT_{h}")
            kT = qkT.tile([2 * D, S], BF16, tag="kT", name=f"kT_{h}")
            headsT[h] = (qT, kT)

        def group(which, g):
            def f():
                src = q_sb if which == 0 else k_sb
                dst = headsT[h][which]
                cols = slice(4 * g * P, 4 * (g + 1) * P)
                tp = tp_pool.tile(
                    [D, 4 * P], BF16, tag="tp", name=f"tp_{h}_{which}_{g}"
                )
                for jj in range(4):
                    nc.tensor.transpose(
                        tp[:, jj * P : (jj + 1) * P], src[:, 4 * g + jj, :], ident_bf
                    )
                nc.vector.tensor_copy(dst[:D, cols], tp)
                nc.sync.dma_start(out=dst[D : 2 * D, cols], in_=dst[:D, cols])

            return f

        first = group(1, 0)
        thunks = [lambda: (alloc(), first())]
        thunks += [group(0, 0)]
        for g in range(1, T // 4):  # interleave K and Q groups
            thunks += [group(1, g), group(0, g)]
        return thunks

    def emit_epilogue_rest(ep):
        # transpose [65, W] -> W/P tiles of [q=128, 65], normalize by the
        # sums row (column 64 after transposing), store.
        h, q0, accs = ep
        ost = outs_pool.tile([P, W // P, D], F32, tag="ost")
        for j in range(W // P):
            ot = tp_pool.tile([P, D + 1], F32, tag="tp")
            nc.tensor.transpose(
                ot, accs[:, j * P : (j + 1) * P], ident_f32[: D + 1, : D + 1]
            )
            nc.vector.reciprocal(ot[:, D : D + 1], ot[:, D : D + 1])
            nc.vector.tensor_scalar_mul(ost[:, j, :], ot[:, 0:D], ot[:, D : D + 1])
        nc.sync.dma_start(
            out=out_h[h, q0 : q0 + W, :].rearrange("(j p) d -> p j d", p=P),
            in_=ost,
        )

    # Flat pipeline over all (head, window, pair) units.  mm2 lags mm1/exp by
    # MM2_LAG units and epilogues lag one more, so every semaphore wait
    # reaching the in-order PE stream is already satisfied and the matmuls
    # chain back-to-back (drains hidden by the next fill).
    MM2_LAG = 2
    NP = T // 2  # k-tile pairs per window
    units = [(h, w, j) for h in range(H) for w in range(NW) for j in range(NP)]
    heads = {0: emit_head_load(0)}
    headsT = {}
    accs_by_window = {}
    pTs = {}
    pending_epi = []
    work_queue = []
    for t in head_prep_thunks(0):
        t()

    def emit_mm2(i):
        h, w, j = units[i]
        acc = accs_by_window[(h, w)]
        v_sb = heads[h][2]
        pT_prev = pTs.pop(i)
        for c, t in ((0, 2 * j), (1, 2 * j + 1)):
            nc.tensor.matmul(
                acc,
                lhsT=v_sb[:, t, :],
                rhs=pT_prev[:, c * W : (c + 1) * W],
                start=(j == 0 and c == 0),
                stop=(j == NP - 1 and c == 1),
            )
        if j == NP - 1:  # window done: drain accumulator, defer the rest
            accs = accs_pool.tile([D + 1, W], F32, tag="accs")
            nc.vector.tensor_copy(accs, acc)
            del accs_by_window[(h, w)]
            pending_epi.append((i + 1, (h, w * W, accs)))

    for i, (h, w, j) in enumerate(units):
        if w == 0 and j == 0 and h > 1:
            del heads[h - 2], headsT[h - 2]
        qT, kT = headsT[h]
        if j == 0:
            accs_by_window[(h, w)] = acc_pool.tile(
                [D + 1, W], F32, tag="acc", name=f"acc_{h}_{w}"
            )
        q0 = w * W
        # one PSUM tile holds S^T for both k-tiles of the pair side by side,
        # written by two concurrently-executing row-tiled matmuls
        st = st_pool.tile([P, 2 * W], F32, tag="st")
        for c, (t, lo) in enumerate(((2 * j, 0), (2 * j + 1, D))):
            nc.tensor.matmul(
                st[:, c * W : (c + 1) * W],
                lhsT=kT[lo : lo + D, t * P : (t + 1) * P],
                rhs=qT[lo : lo + D, q0 : q0 + W],
                start=True,
                stop=True,
            )
        pT = ppool.tile([P, 2 * W], BF16, tag="pT")
        nc.scalar.activation(
            out=pT, in_=st, func=mybir.ActivationFunctionType.Exp, scale=SCALE
        )
        pTs[i] = pT
        if i >= MM2_LAG:
            emit_mm2(i - MM2_LAG)
        while pending_epi and pending_epi[0][0] <= i - MM2_LAG:
            emit_epilogue_rest(pending_epi.pop(0)[1])
        if j == 2 and w == 0 and h + 1 < H:
            heads[h + 1] = emit_head_load(h + 1)
        if j == 0 and w == 1 and h + 1 < H:
            work_queue.extend(head_prep_thunks(h + 1))
        if work_queue:
            work_queue.pop(0)()
    for i in range(len(units) - MM2_LAG, len(units)):
        emit_mm2(i)
    for _, ep in pending_epi:
        emit_epilogue_rest(ep)


def build_nc():
    nc = bacc.Bacc("TRN2", target_bir_lowering=False, debug=False, num_devices=N_CORES)
    q = nc.declare_dram_parameter("q", [H, S, D], F32, isOutput=False)
    k = nc.declare_dram_parameter("k", [H, S, D], F32, isOutput=False)
    v = nc.declare_dram_parameter("v", [H, S, D], F32, isOutput=False)
    mask = nc.declare_dram_parameter("mask", [S], I32, isOutput=False)
    out = nc.declare_dram_parameter("out", [H, S, D], F32, isOutput=True)
    from contextlib import ExitStack

    with tile.TileContext(nc) as tc, ExitStack() as ctx:
        emit_core_program(ctx, nc, tc, q.ap(), k.ap(), v.ap(), mask.ap(), out.ap())
    nc.compile()
    return nc


_NC_CACHE = []


def get_nc():
    if not _NC_CACHE:
        _NC_CACHE.append(build_nc())
    return _NC_CACHE[0]


def make_in_maps(q, k, v, mask):
    """Shard full [B,NH,S,D] inputs into per-core input maps (8 heads/core)."""
    qf = np.asarray(q, dtype=np.float32).reshape(B * NH, S, D)
    kf = np.asarray(k, dtype=np.float32).reshape(B * NH, S, D)
    vf = np.asarray(v, dtype=np.float32).reshape(B * NH, S, D)
    mf = np.asarray(mask, dtype=np.int32).reshape(B, S)
    in_maps = []
    for c in range(N_CORES):
        lo = c * H
        in_maps.append(
            {
                "q": np.ascontiguousarray(qf[lo : lo + H]),
                "k": np.ascontiguousarray(kf[lo : lo + H]),
                "v": np.ascontiguousarray(vf[lo : lo + H]),
                # heads lo..lo+H-1 all belong to batch lo // NH
                "mask": np.ascontiguousarray(mf[lo // NH]),
            }
        )
    return in_maps


def kernel(q, k, v, mask):
    from concourse.bass_utils import run_bass_kernel_spmd

    nc = get_nc()
    in_maps = make_in_maps(q, k, v, mask)
    res = run_bass_kernel_spmd(nc, in_maps, list(range(N_CORES))).results
    out = np.concatenate([res[c]["out"] for c in range(N_CORES)], axis=0)
    return out.reshape(B, NH, S, D)


if __name__ == "__main__":
    nc = build_nc()
    print("built ok")


# revision 34
# speedup vs baseline: 2.2151x; 1.0133x over previous
"""Trainium2 Bass kernel for nn_BaseAttention (B=4, H=16, S=2048, D=64, key-mask).

Strategy (8 NeuronCores, batch*head sharded, 8 heads per core; each core's 8
heads happen to share one batch's mask):
  For each head (Q,K,V: [S,D] f32, mask: [S] int 0/1):
    - Load Q,K,V with fp32->bf16 cast during DMA (SWDGE).
    - PE-transpose Q,K tiles -> Q^T, K^T [64, S] bf16, duplicated onto
      partitions 64-127 so mm1 can run two k-tiles concurrently in the two
      row halves of the PE array (row tiling, K=64 each).
    - Scores transposed: S^T[k, q] = K @ Q^T, fp32 PSUM, one [128, 2*512]
      tile per k-tile pair; one ScalarE pass computes P^T = Exp(S^T/8).
      No max-subtraction: scores are ~N(0,1) so exp cannot overflow, and no
      additive mask: the key mask is applied by zeroing masked rows of
      V' = [V | ones] (out = sum_k exp(s_k) V'[k] makes that exactly
      equivalent, including the softmax denominator in the ones column).
    - mm2 accumulates out'^T [65, q] over k; the sums row is the denominator.
    - Reciprocal of sums, PE-transpose [65, q] -> [q, 65], scale, store.
  Emission is a flat software pipeline over (head, window, k-pair) units with
  mm2 and epilogues lagging 2 units, so the in-order PE stream never reaches
  an unmet semaphore and matmuls chain back-to-back.

Self-contained: hardcodes shapes; imports concourse from /opt/trn_rl_repo.
"""

import sys

if "/opt/trn_rl_repo" not in sys.path:
    sys.path.insert(0, "/opt/trn_rl_repo")

import numpy as np

import concourse.bass as bass
import concourse.mybir as mybir
import concourse.tile as tile
from concourse import bacc
from concourse.masks import make_identity

F32 = mybir.dt.float32
BF16 = mybir.dt.bfloat16
I32 = mybir.dt.int32

N_CORES = 8
B, NH, S, D = 4, 16, 2048, 64
H = (B * NH) // N_CORES  # heads per core = 8
P = 128                  # partitions / k-tile size
T = S // P               # 16 k-tiles per head
W = 512                  # q-window width (= fp32 PSUM bank limit per matmul)
NW = S // W              # 4 q-windows per head
SCALE = 1.0 / 8.0        # 1/sqrt(D)


def emit_core_program(ctx, nc, tc, q_h, k_h, v_h, mask_h, out_h):
    """Emit the per-core Tile program. q/k/v/out: DRAM APs [H, S, D]; mask: [S] i32."""
    pool = lambda *a, **kw: ctx.enter_context(tc.tile_pool(*a, **kw))
    singles = pool(name="singles", bufs=1)
    ld = pool(name="ld", bufs=2)            # SBUF head staging (bf16)
    qkT = pool(name="qkT", bufs=2)          # SBUF Q^T/K^T (both row halves)
    ppool = pool(name="p", bufs=5)          # SBUF P^T tiles (lagged mm2)
    accs_pool = pool(name="accs", bufs=2)   # SBUF drained accumulators
    outs_pool = pool(name="outs", bufs=2)   # SBUF output staging
    st_pool = pool(name="st", bufs=2, space="PSUM")    # S^T pair tiles (2 banks ea)
    acc_pool = pool(name="acc", bufs=2, space="PSUM")  # out'^T accum (1 bank ea)
    tp_pool = pool(name="tp", bufs=2, space="PSUM")    # transposes (1 bank ea)

    ident_bf = singles.tile([P, P], BF16)
    make_identity(nc, ident_bf)
    ident_f32 = singles.tile([P, P], F32)
    make_identity(nc, ident_f32)

    # mask [S] i32 -> om [128, T] f32 = 1 - mask  (om[p, t] = keep key t*128+p)
    mask_i = singles.tile([P, T], I32)
    nc.sync.dma_start(out=mask_i, in_=mask_h.rearrange("(t p) -> p t", p=P))
    om = singles.tile([P, T], F32)
    nc.vector.tensor_scalar(
        om, mask_i, -1.0, 1.0, mybir.AluOpType.mult, mybir.AluOpType.add
    )

    def emit_head_load(h):
        q_sb = ld.tile([P, T, D], BF16, tag="q_sb", name=f"q_sb_{h}")
        nc.gpsimd.dma_start(out=q_sb, in_=q_h[h].rearrange("(t p) d -> p t d", p=P))
        k_sb = ld.tile([P, T, D], BF16, tag="k_sb", name=f"k_sb_{h}")
        nc.gpsimd.dma_start(out=k_sb, in_=k_h[h].rearrange("(t p) d -> p t d", p=P))
        v_sb = ld.tile([P, T, D + 1], BF16, tag="v_sb", name=f"v_sb_{h}")
        nc.gpsimd.dma_start(
            out=v_sb[:, :, 0:D], in_=v_h[h].rearrange("(t p) d -> p t d", p=P)
        )
        nc.vector.memset(v_sb[:, :, D : D + 1], 1.0)
        # zero masked key rows of V' (applies the mask): one multiply with
        # om broadcast along d via a zero-stride AP dim
        om_b = bass.AP(tensor=om.tensor, offset=om.offset, ap=om.ap + [[0, D + 1]])
        nc.vector.tensor_mul(v_sb, v_sb, om_b)
        return q_sb, k_sb, v_sb

    def head_prep_thunks(h):
        # Q^T / K^T [64, S] bf16, each 512-col slice duplicated onto
        # partitions 64-127 right after it is built (SBUF->SBUF DMA) so mm1
        # row-tile pairs never wait long on a duplicate.  Split into small
        # thunks so the PE work spreads across many pipeline units.
        q_sb, k_sb, _ = heads[h]

        def alloc():
            qT = qkT.tile([2 * D, S], BF16, tag="qT", name=f"qT_{h}")
            kT = qkT.tile([2 * D, S], BF16, tag="kT", name=f"kT_{h}")
            headsT[h] = (qT, kT)

        def group(which, g):
            def f():
                src = q_sb if which == 0 else k_sb
                dst = headsT[h][which]
                cols = slice(4 * g * P, 4 * (g + 1) * P)
                tp = tp_pool.tile(
                    [D, 4 * P], BF16, tag="tp", name=f"tp_{h}_{which}_{g}"
                )
                for jj in range(4):
                    nc.tensor.transpose(
                        tp[:, jj * P : (jj + 1) * P], src[:, 4 * g + jj, :], ident_bf
                    )
                nc.vector.tensor_copy(dst[:D, cols], tp)
                nc.sync.dma_start(out=dst[D : 2 * D, cols], in_=dst[:D, cols])

            return f

        first = group(1, 0)
        thunks = [lambda: (alloc(), first())]
        thunks += [group(0, 0)]
        for g in range(1, T // 4):  # interleave K and Q groups
            thunks += [group(1, g), group(0, g)]
        return thunks

    def emit_epilogue_rest(ep):
        # transpose [65, W] -> W/P tiles of [q=128, 65], normalize by the
        # sums row (column 64 after transposing), store.
        h, q0, accs = ep
        ost = outs_pool.tile([P, W // P, D], F32, tag="ost")
        for j in range(W // P):
            ot = tp_pool.tile([P, D + 1], F32, tag="tp")
            nc.tensor.transpose(
                ot, accs[:, j * P : (j + 1) * P], ident_f32[: D + 1, : D + 1]
            )
            nc.vector.reciprocal(ot[:, D : D + 1], ot[:, D : D + 1])
            nc.vector.tensor_scalar_mul(ost[:, j, :], ot[:, 0:D], ot[:, D : D + 1])
        nc.sync.dma_start(
            out=out_h[h, q0 : q0 + W, :].rearrange("(j p) d -> p j d", p=P),
            in_=ost,
        )

    # Flat pipeline over all (head, window, pair) units.  mm2 lags mm1/exp by
    # MM2_LAG units and epilogues lag one more, so every semaphore wait
    # reaching the in-order PE stream is already satisfied and the matmuls
    # chain back-to-back (drains hidden by the next fill).
    MM2_LAG = 3
    NP = T // 2  # k-tile pairs per window
    units = [(h, w, j) for h in range(H) for w in range(NW) for j in range(NP)]
    heads = {0: emit_head_load(0)}
    headsT = {}
    accs_by_window = {}
    pTs = {}
    pending_epi = []
    work_queue = []
    for t in head_prep_thunks(0):
        t()

    def emit_mm2(i):
        h, w, j = units[i]
        acc = accs_by_window[(h, w)]
        v_sb = heads[h][2]
        pT_prev = pTs.pop(i)
        for c, t in ((0, 2 * j), (1, 2 * j + 1)):
            nc.tensor.matmul(
                acc,
                lhsT=v_sb[:, t, :],
                rhs=pT_prev[:, c * W : (c + 1) * W],
                start=(j == 0 and c == 0),
                stop=(j == NP - 1 and c == 1),
            )
        if j == NP - 1:  # window done: drain accumulator, defer the rest
            accs = accs_pool.tile([D + 1, W], F32, tag="accs")
            nc.vector.tensor_copy(accs, acc)
            del accs_by_window[(h, w)]
            pending_epi.append((i + 1, (h, w * W, accs)))

    for i, (h, w, j) in enumerate(units):
        if w == 0 and j == 0 and h > 1:
            del heads[h - 2], headsT[h - 2]
        qT, kT = headsT[h]
        if j == 0:
            accs_by_window[(h, w)] = acc_pool.tile(
                [D + 1, W], F32, tag="acc", name=f"acc_{h}_{w}"
            )
        q0 = w * W
        # one PSUM tile holds S^T for both k-tiles of the pair side by side,
        # written by two concurrently-executing row-tiled matmuls
        st = st_pool.tile([P, 2 * W], F32, tag="st")
        for c, (t, lo) in enumerate(((2 * j, 0), (2 * j + 1, D))):
            nc.tensor.matmul(
                st[:, c * W : (c + 1) * W],
                lhsT=kT[lo : lo + D, t * P : (t + 1) * P],
                rhs=qT[lo : lo + D, q0 : q0 + W],
                start=True,
                stop=True,
            )
        pT = ppool.tile([P, 2 * W], BF16, tag="pT")
        nc.scalar.activation(
            out=pT, in_=st, func=mybir.ActivationFunctionType.Exp, scale=SCALE
        )
        pTs[i] = pT
        if i >= MM2_LAG:
            emit_mm2(i - MM2_LAG)
        while pending_epi and pending_epi[0][0] <= i - MM2_LAG:
            emit_epilogue_rest(pending_epi.pop(0)[1])
        if j == 2 and w == 0 and h + 1 < H:
            heads[h + 1] = emit_head_load(h + 1)
        if j == 0 and w == 1 and h + 1 < H:
            work_queue.extend(head_prep_thunks(h + 1))
        if work_queue:
            work_queue.pop(0)()
    for i in range(len(units) - MM2_LAG, len(units)):
        emit_mm2(i)
    for _, ep in pending_epi:
        emit_epilogue_rest(ep)


def build_nc():
    nc = bacc.Bacc("TRN2", target_bir_lowering=False, debug=False, num_devices=N_CORES)
    q = nc.declare_dram_parameter("q", [H, S, D], F32, isOutput=False)
    k = nc.declare_dram_parameter("k", [H, S, D], F32, isOutput=False)
    v = nc.declare_dram_parameter("v", [H, S, D], F32, isOutput=False)
    mask = nc.declare_dram_parameter("mask", [S], I32, isOutput=False)
    out = nc.declare_dram_parameter("out", [H, S, D], F32, isOutput=True)
    from contextlib import ExitStack

    with tile.TileContext(nc) as tc, ExitStack() as ctx:
        emit_core_program(ctx, nc, tc, q.ap(), k.ap(), v.ap(), mask.ap(), out.ap())
    nc.compile()
    return nc


_NC_CACHE = []


def get_nc():
    if not _NC_CACHE:
        _NC_CACHE.append(build_nc())
    return _NC_CACHE[0]


def make_in_maps(q, k, v, mask):
    """Shard full [B,NH,S,D] inputs into per-core input maps (8 heads/core)."""
    qf = np.asarray(q, dtype=np.float32).reshape(B * NH, S, D)
    kf = np.asarray(k, dtype=np.float32).reshape(B * NH, S, D)
    vf = np.asarray(v, dtype=np.float32).reshape(B * NH, S, D)
    mf = np.asarray(mask, dtype=np.int32).reshape(B, S)
    in_maps = []
    for c in range(N_CORES):
        lo = c * H
        in_maps.append(
            {
                "q": np.ascontiguousarray(qf[lo : lo + H]),
                "k": np.ascontiguousarray(kf[lo : lo + H]),
                "v": np.ascontiguousarray(vf[lo : lo + H]),
                # heads lo..lo+H-1 all belong to batch lo // NH
                "mask": np.ascontiguousarray(mf[lo // NH]),
            }
        )
    return in_maps


def kernel(q, k, v, mask):
    from concourse.bass_utils import run_bass_kernel_spmd

    nc = get_nc()
    in_maps = make_in_maps(q, k, v, mask)
    res = run_bass_kernel_spmd(nc, in_maps, list(range(N_CORES))).results
    out = np.concatenate([res[c]["out"] for c in range(N_CORES)], axis=0)
    return out.reshape(B, NH, S, D)


if __name__ == "__main__":
    nc = build_nc()
    print("built ok")
